# revision 1
# baseline (speedup 1.0000x reference)
"""Trainium2 Bass kernel for nn_Block_22497038696617 (dense transformer block).

Block: pre-LN attention with policy-masked softmax + pre-LN MLP (exact GELU).
  B=2, N=2048, C=768, H=12 heads x 64, HID=3072, fp32 in/out.

Sharding (8 cores, zero cross-core communication, single SPMD launch):
  core c -> batch b = c//4, query block qoff = (c%4)*512.
  Each core computes LN1 + K/V for the full (compacted) sequence of its
  batch, Q/attention/proj/MLP for its own 512 query rows, and writes its
  [C, 512] output slice (transposed). Host gathers + transposes.

Key compaction: attention is permutation-invariant over keys, and a key with
policy 0 contributes ~0 everywhere except its own query's diagonal. Each
core's key axis is [own 512 queries, unmasked other keys, pad], so the
diagonal exception lives in k-tiles 0..3 at column offset t*128 and the SPMD
program is identical on all cores.

v2 (this file) vs the f32r baseline:
  * all matmul operands in bf16 (weights converted on host): halves DMA and
    SBUF, enables the PE fast-weight-load path (fp32 LDWEIGHTS was ~50% of
    PE occupancy), doubles DVE throughput on masks/copies. PSUM stays f32.
    Tolerance is 2e-2; bf16 keeps us ~1e-3.
  * the diagonal "always attend to self" fix is two extra PE matmuls per
    diagonal k-tile (diag(-lnp/scale) @ eye accumulated into the S PSUM
    group) instead of a 5-op DVE mask chain: exp(scale*(s + 400*(1-pol)) +
    lnp) == exp(scale*s) on the diagonal. The ACT exp with per-key bias
    lnp = ln(policy) (0 / -50) handles everything else, so every k-tile is
    uniform and the softmax costs exactly one ACT instruction per tile.
  * softmax 1/denominator: denominator rows (from the ones-column in V) are
    collected into one [12, 512] tile via tiny SBUF->SBUF DMAs and inverted
    with ONE Ln + ONE Exp call (the baseline paid 4 ACT calls per head pair,
    ~17us on the ACT critical path).
  * proj / LN2 / MLP all run in the transposed [C, queries] layout:
    proj swaps stationary/moving so x_res comes out as [c, q], the residual
    adds use the already-transposed x (no x_own load), LN2 stats are
    ones-matmuls like LN1, and the 24 PE transposes + copies of the h2
    tensor are gone. fc1/fc2 are interleaved per 128-hid block so the PE
    never drains between them, and the final residual add happens on-device
    (single [C, 512] f32 output per core).
  * K weights load once (resident in bf16), K/stats/hl work is clipped to
    the real key count (kt_b*128) instead of the 512-padded kpad.
"""

from contextlib import ExitStack

import numpy as np
import ml_dtypes

import concourse.bacc as bacc
import concourse.mybir as mybir
import concourse.tile as tile
from concourse.bass_utils import run_bass_kernel_spmd

f32 = mybir.dt.float32
f32r = mybir.dt.float32r
bf16 = mybir.dt.bfloat16
AF = mybir.ActivationFunctionType
OP = mybir.AluOpType

B, N, C = 2, 2048, 768
H, HD = 12, 64
HID = 3072
NCORES = 8
QB = 512                 # own query rows per core
CT = C // 128            # 6 c-tiles
FT = C // 128            # 6 f-tiles (H*HD == C)
HB = HID // 128          # 24 hid-tiles
SCALE = HD ** -0.5
LN_EPS = 1e-5
POL_EPS = 1e-6
MASK_NEG = -50.0

TRACE = False            # set True by the dev harness for profiling runs
TRACE_KWARGS = {}
LAST_RESULTS = None      # BassKernelResults of the last run (for timing)

_prog_cache = {}


def _build_program(ln1_triv, ln2_triv, projb_triv, kpad, kt_b):
    # kt_b = number of k-tiles containing any real key; K/V/S/O work beyond
    # kt_b*128 columns is skipped entirely
    kt_n = kpad // 128
    kq_n = (kt_b * 128 + 511) // 512     # 512-wide key chunks with real keys
    kb = kt_b * 128                      # real-key column count (128-aligned)
    nc = bacc.Bacc("TRN2", target_bir_lowering=False, debug=False,
                   num_devices=NCORES)

    # ---- DRAM I/O ----
    xT_d = nc.dram_tensor("xT", [C, kpad], bf16, kind="ExternalInput")
    xownT_d = nc.dram_tensor("xownT", [C, QB], f32, kind="ExternalInput")
    lnp_d = nc.dram_tensor("lnp", [128, kt_n], f32, kind="ExternalInput")
    # weight packs: [fj, p, ci*128+f] so [fj][:, ci*128:(ci+1)*128] is the
    # stationary tile for c-tile ci (contiguous lines)
    wq_d = nc.dram_tensor("wq_packT", [FT, 128, CT * 128], bf16,
                          kind="ExternalInput")
    wk_d = nc.dram_tensor("wk_packT", [FT, 128, CT * 128], bf16,
                          kind="ExternalInput")
    wv_d = nc.dram_tensor("wv_packT", [128, CT * C], bf16,
                          kind="ExternalInput")
    projw_d = nc.dram_tensor("projwT", [C, C], bf16, kind="ExternalInput")
    fc1w_d = nc.dram_tensor("fc1w_pack", [CT, 6, 128, 512], bf16,
                            kind="ExternalInput")
    fc2w_d = nc.dram_tensor("fc2wT", [HID, C], bf16, kind="ExternalInput")
    fc1b_d = nc.dram_tensor("fc1b", [128, HB], f32, kind="ExternalInput")
    eye_d = nc.dram_tensor("eye", [128, 128], bf16, kind="ExternalInput")
    if not ln1_triv:
        ln1gb_d = nc.dram_tensor("ln1gb", [128, 2 * CT], f32,
                                 kind="ExternalInput")
    if not ln2_triv:
        ln2gb_d = nc.dram_tensor("ln2gb", [128, 2 * CT], f32,
                                 kind="ExternalInput")
    if not projb_triv:
        projb_d = nc.dram_tensor("projb", [128, CT], f32,
                                 kind="ExternalInput")
    yT_d = nc.dram_tensor("yT", [C, QB], bf16, kind="ExternalOutput")

    with tile.TileContext(nc) as tc, ExitStack() as ctx:
        # ---------------- constants + whole-kernel persistents --------------
        pG = ctx.enter_context(tc.tile_pool(name="pG", bufs=1))
        eye_sb = pG.tile([128, 128], bf16, name="eye_sb")
        nc.gpsimd.dma_start(out=eye_sb, in_=eye_d.ap())
        lnp_sb = pG.tile([128, kt_n], f32, name="lnp_sb")
        nc.gpsimd.dma_start(out=lnp_sb, in_=lnp_d.ap())
        fc1b_sb = pG.tile([128, HB], f32, name="fc1b_sb")
        nc.gpsimd.dma_start(out=fc1b_sb, in_=fc1b_d.ap())
        ones_bf = pG.tile([128, 1], bf16, name="ones_bf")
        nc.vector.memset(ones_bf, 1.0)
        ones_fr = pG.tile([128, 1], f32r, name="ones_fr")
        nc.vector.memset(ones_fr.bitcast(f32), 1.0)
        ones_row = pG.tile([1, 128], bf16, name="ones_row")
        nc.vector.memset(ones_row, 1.0)
        # diagonal-fix stationary tiles: diagd[t][k, j] = eye * (-lnp/SCALE)
        diagd = []
        for t in range(4):
            d_ = pG.tile([128, 128], bf16, name=f"diagd{t}")
            nc.vector.tensor_scalar(d_, eye_sb, lnp_sb[:, t:t + 1],
                                    -1.0 / SCALE, op0=OP.mult, op1=OP.mult)
            diagd.append(d_)
        # attention output, transposed, per head-pair: OTp[j] rows = features
        # of heads (2j, 2j+1), cols = own queries
        OTp = [pG.tile([128, QB], bf16, name=f"otp{j}") for j in range(FT)]
        # unnormalized per-head attention output + denominator row
        o_keep = [pG.tile([HD + 1, QB], bf16, name=f"okeep{h}")
                  for h in range(H)]
        # denominator rows grouped by normalization batch (engines can only
        # address APs based at partition 0, so one tile per batch; heads
        # 10/11 are normalized individually after the pair loop)
        dall = [pG.tile([6, QB], bf16, name="dall0"),
                pG.tile([6, QB], bf16, name="dall1"),
                pG.tile([1, QB], bf16, name="dall2"),
                pG.tile([1, QB], bf16, name="dall3")]
        _dall_slot = lambda h: ((0, h) if h < 6 else
                                (1, h - 6) if h < 10 else (2 + h - 10, 0))
        # attention-residual (transposed) lives here so proj can write it and
        # phase C can read it
        # f32r so the LN2 stats matmuls can consume it directly (the BIR
        # verifier requires f32r matmul inputs to be rounded at the producer)
        x_resT = [pG.tile([128, QB], f32r, name=f"xrest{cj}")
                  for cj in range(CT)]

        # ======================= phase A + B scope ==========================
        with tc.tile_pool(name="pAB", bufs=1) as pAB:
            KTp = [pAB.tile([128, kb], bf16, name=f"ktp{j}") for j in range(FT)]
            QTp = [pAB.tile([128, QB], bf16, name=f"qtp{j}") for j in range(FT)]
            vpad = [pAB.tile([128, H, HD + 1], bf16, name=f"vpad{t}")
                    for t in range(kt_b)]
            # resident K weights (single DMA, first on the gpsimd queue so
            # kvq(0) is never DMA-gated), then V; Q weights stream in pA
            wk_sb = [pAB.tile([128, CT * 128], bf16, name=f"wk{fj}")
                     for fj in range(FT)]
            for fj in range(FT):
                nc.gpsimd.dma_start(out=wk_sb[fj], in_=wk_d.ap()[fj])
            wv_sb = pAB.tile([128, CT, C], bf16, name="wv_sb")
            nc.gpsimd.dma_start(
                out=wv_sb.rearrange("p a b -> p (a b)"), in_=wv_d.ap())
            # pre-warm the partition_broadcast ucode library (~10us invisible
            # IRAM load that blocks the gpsimd queue): after the weight DMAs,
            # long before phase B needs the first real broadcast
            bwarm = pAB.tile([2, 1], f32, name="bwarm")
            nc.gpsimd.partition_broadcast(bwarm, ones_fr.bitcast(f32)[0:1, :])

            # --------------- phase A: LN1 + QKV projections -----------------
            # Software-pipelined: chunk q's LN stats/apply overlap chunk
            # q-1's K/V/Q matmuls.
            with tc.tile_pool(name="pA", bufs=1) as pA, \
                 tc.tile_pool(name="psA", bufs=1, space="PSUM") as psA:
                if not ln1_triv:
                    ln1gb_sb = pA.tile([128, 2 * CT], f32, name="ln1gb_sb")
                    nc.sync.dma_start(out=ln1gb_sb, in_=ln1gb_d.ap())

                def ln_loads_stats(qr):
                    """x.T load (one 3-D DMA) + stats matmuls for one chunk."""
                    s0 = qr * 512
                    kw = min(512, kb - s0)
                    xtt = pA.tile([128, CT, 512], bf16, name="xt", tag="xt",
                                  bufs=2)
                    nc.sync.dma_start(
                        out=xtt[:, :, 0:kw],
                        in_=xT_d.ap()[:, s0:s0 + kw].rearrange(
                            "(a p) k -> p a k", p=128))
                    xt = [xtt[:, ci, :] for ci in range(CT)]
                    # stats via ones-matmuls (sum over c = partition dim)
                    ps_mean = psA.tile([1, 512], f32, name="ps_mean",
                                       tag="psmean", bufs=1)
                    ps_sq = psA.tile([1, 512], f32, name="ps_sq",
                                     tag="pssq", bufs=1)
                    for ci in range(CT):
                        nc.tensor.matmul(ps_mean[:, 0:kw], ones_bf,
                                         xt[ci][:, 0:kw],
                                         start=(ci == 0), stop=(ci == CT - 1))
                    for ci in range(CT):
                        xsq = pA.tile([128, 512], bf16, name="xsq", tag="xsq",
                                      bufs=2)
                        nc.vector.tensor_mul(xsq[:, 0:kw], xt[ci][:, 0:kw],
                                             xt[ci][:, 0:kw])
                        nc.tensor.matmul(ps_sq[:, 0:kw], ones_bf,
                                         xsq[:, 0:kw],
                                         start=(ci == 0), stop=(ci == CT - 1))
                    return xt, ps_mean, ps_sq, kw

                def ln_rows_hl(qr, stage):
                    """LN1 row stats -> broadcast -> h_ln.T build (bf16)."""
                    xt, ps_mean, ps_sq, kw = stage
                    def row(nm, dt=f32):
                        return pA.tile([1, 512], dt, name=nm, tag="rows",
                                       bufs=4)
                    mrow = row("mrow")
                    nc.vector.tensor_scalar_mul(mrow[:, 0:kw],
                                                ps_mean[:, 0:kw], 1.0 / C)
                    ve = row("ve")
                    nc.vector.tensor_scalar(ve[:, 0:kw], ps_sq[:, 0:kw],
                                            1.0 / C, LN_EPS,
                                            op0=OP.mult, op1=OP.add)
                    m2 = row("m2")
                    nc.vector.tensor_mul(m2[:, 0:kw], mrow[:, 0:kw],
                                         mrow[:, 0:kw])
                    nc.vector.tensor_sub(ve[:, 0:kw], ve[:, 0:kw],
                                         m2[:, 0:kw])
                    nc.scalar.activation(ve[:, 0:kw], ve[:, 0:kw], AF.Ln)
                    r0 = row("r0")
                    nc.scalar.activation(r0[:, 0:kw], ve[:, 0:kw], AF.Exp,
                                         scale=-0.5)
                    m16 = pA.tile([1, 512], bf16, name="m16", tag="rows16",
                                  bufs=2)
                    nc.vector.tensor_copy(m16[:, 0:kw], mrow[:, 0:kw])
                    r16 = pA.tile([1, 512], bf16, name="r16", tag="rows16",
                                  bufs=2)
                    nc.vector.tensor_copy(r16[:, 0:kw], r0[:, 0:kw])
                    # broadcast m/r across partitions on the PE (ones-row
                    # outer product) -- the gpsimd broadcast ucode costs a
                    # ~10us queue-blocking library load on first use
                    ps_bc = psA.tile([128, 1024], f32, name="ps_bc",
                                     tag="psbc", bufs=1)
                    nc.tensor.matmul(ps_bc[:, 0:kw], ones_row, m16[:, 0:kw],
                                     start=True, stop=True)
                    nc.tensor.matmul(ps_bc[:, 512:512 + kw], ones_row,
                                     r16[:, 0:kw], start=True, stop=True)
                    bc_m = pA.tile([128, 512], bf16, name="bc_m", tag="bc_m",
                                   bufs=2)
                    nc.vector.tensor_copy(bc_m[:, 0:kw], ps_bc[:, 0:kw])
                    bc_r = pA.tile([128, 512], bf16, name="bc_r", tag="bc_r",
                                   bufs=2)
                    nc.vector.tensor_copy(bc_r[:, 0:kw],
                                          ps_bc[:, 512:512 + kw])
                    hl = []
                    for ci in range(CT):
                        h_ = pA.tile([128, 512], bf16, name="hl", tag="hl",
                                     bufs=12)
                        nc.vector.tensor_sub(h_[:, 0:kw], xt[ci][:, 0:kw],
                                             bc_m[:, 0:kw])
                        nc.vector.tensor_tensor(
                            out=h_[:, 0:kw], in0=h_[:, 0:kw],
                            in1=bc_r[:, 0:kw], op=OP.mult)
                        if not ln1_triv:
                            nc.vector.tensor_scalar(
                                h_[:, 0:kw], h_[:, 0:kw],
                                ln1gb_sb[:, ci:ci + 1],
                                ln1gb_sb[:, CT + ci:CT + ci + 1],
                                op0=OP.mult, op1=OP.add)
                        hl.append(h_)
                    return hl, kw

                def kvq_stage(qr, hlkw):
                    """K/V (+Q for chunk 0) matmuls for one chunk."""
                    hl, kw = hlkw
                    s0 = qr * 512
                    for fj in range(FT):
                        psk = psA.tile([128, 512], f32, name="psk",
                                       tag="pskv", bufs=3)
                        for ci in range(CT):
                            nc.tensor.matmul(
                                psk[:, 0:kw],
                                wk_sb[fj][:, ci * 128:(ci + 1) * 128],
                                hl[ci][:, 0:kw],
                                start=(ci == 0), stop=(ci == CT - 1))
                        nc.vector.tensor_copy(KTp[fj][:, s0:s0 + kw],
                                              psk[:, 0:kw])
                    for si in range(kw // 128):
                        st = qr * 4 + si
                        for fc in range(2):
                            f0 = fc * 512
                            wsz = 512 if fc == 0 else 256
                            psv = psA.tile([128, 512], f32, name="psv",
                                           tag="pskv", bufs=3)
                            for ci in range(CT):
                                nc.tensor.matmul(
                                    psv[:, 0:wsz],
                                    hl[ci][:, si * 128:(si + 1) * 128],
                                    wv_sb[:, ci, f0:f0 + wsz],
                                    start=(ci == 0), stop=(ci == CT - 1))
                            nh = wsz // HD
                            h0 = 0 if fc == 0 else 8
                            nc.vector.tensor_copy(
                                vpad[st][:, h0:h0 + nh, 0:HD],
                                psv[:, 0:wsz].rearrange(
                                    "p (h d) -> p h d", h=nh))
                        nc.vector.memset(vpad[st][:, :, HD], 1.0)
                    if qr == 0:
                        # own queries are keys 0:512 => Q.T from chunk 0
                        for fj in range(FT):
                            wq_t = pA.tile([128, CT * 128], bf16, name="wq",
                                           tag="wq", bufs=2)
                            nc.sync.dma_start(out=wq_t, in_=wq_d.ap()[fj])
                            psq = psA.tile([128, 512], f32, name="psq",
                                           tag="pskv", bufs=3)
                            for ci in range(CT):
                                nc.tensor.matmul(
                                    psq,
                                    wq_t[:, ci * 128:(ci + 1) * 128],
                                    hl[ci],
                                    start=(ci == 0), stop=(ci == CT - 1))
                            nc.vector.tensor_copy(QTp[fj], psq)

                # 2-deep software pipeline (see baseline)
                stage = ln_loads_stats(0)
                hl_prev = ln_rows_hl(0, stage)
                for qr in range(1, kq_n):
                    stage = ln_loads_stats(qr)
                    kvq_stage(qr - 1, hl_prev)
                    hl_prev = ln_rows_hl(qr, stage)
                kvq_stage(kq_n - 1, hl_prev)

            # --------------- phase B: attention (head pairs) ----------------
            with tc.tile_pool(name="pB", bufs=1) as pB, \
                 tc.tile_pool(name="psB", bufs=1, space="PSUM") as psB:
                # proj inputs live in pAB (they outlive the pB scope: proj
                # runs in the psP scope after attention)
                projw = [pAB.tile([128, C], bf16, name=f"pjw{fj}")
                         for fj in range(FT)]
                for fj in range(FT):
                    nc.sync.dma_start(
                        out=projw[fj],
                        in_=projw_d.ap()[fj * 128:(fj + 1) * 128, :])
                xownT = [pAB.tile([128, QB], f32, name=f"xownt{cj}")
                         for cj in range(CT)]
                for cj in range(CT):
                    nc.sync.dma_start(
                        out=xownT[cj],
                        in_=xownT_d.ap()[cj * 128:(cj + 1) * 128, :])
                if not projb_triv:
                    projb_sb = pAB.tile([128, CT], f32, name="projb_sb")
                    nc.sync.dma_start(out=projb_sb, in_=projb_d.ap())
                # heads 0-5 after pair 3, 6-9 after pair 4 (emitted in pair
                # 5's epilogue slot would be too late; key them to jp=4), and
                # 10-11 right at the end. Batches never cross a dall tile.
                norm_sched = {3: ((0, 6),), 4: ((6, 4),)}
                for jp in range(H // 2):
                    h0, h1 = 2 * jp, 2 * jp + 1
                    ps_o0 = psB.tile([HD + 1, QB], f32, name="ps_o0",
                                     tag="pso", bufs=4)
                    ps_o1 = psB.tile([HD + 1, QB], f32, name="ps_o1",
                                     tag="pso", bufs=4)

                    def o_mms(ti, t, p_t):
                        nc.tensor.matmul(ps_o0, vpad[t][:, h0, :],
                                         p_t[:, 0:QB],
                                         start=(ti == 0), stop=(ti == kt_b - 1),
                                         skip_group_check=True)
                        nc.tensor.matmul(ps_o1, vpad[t][:, h1, :],
                                         p_t[:, QB:2 * QB],
                                         start=(ti == 0), stop=(ti == kt_b - 1),
                                         skip_group_check=True)

                    prev = None
                    # diagonal tiles (2 extra addend matmuls each) last, so
                    # the pair's first exp isn't delayed during pipeline fill
                    for ti, t in enumerate(list(range(4, kt_b)) + [0, 1, 2, 3]):
                        ps_s = psB.tile([128, 2 * QB], f32, name="ps_s",
                                        tag="pss", bufs=2)
                        nc.tensor.matmul(
                            ps_s[:, 0:QB],
                            KTp[jp][0:64, t * 128:(t + 1) * 128],
                            QTp[jp][0:64, :],
                            start=True, stop=(t >= 4), skip_group_check=True)
                        nc.tensor.matmul(
                            ps_s[:, QB:2 * QB],
                            KTp[jp][64:128, t * 128:(t + 1) * 128],
                            QTp[jp][64:128, :],
                            start=True, stop=(t >= 4), skip_group_check=True)
                        if t < 4:
                            # diagonal fix: +(-lnp/SCALE) on the self column
                            off = t * 128
                            nc.tensor.matmul(
                                ps_s[:, off:off + 128], diagd[t], eye_sb,
                                start=False, stop=True, skip_group_check=True)
                            nc.tensor.matmul(
                                ps_s[:, QB + off:QB + off + 128], diagd[t],
                                eye_sb,
                                start=False, stop=True, skip_group_check=True)
                        p_t = pB.tile([128, 2 * QB], bf16, name="p_t",
                                      tag="pt", bufs=4)
                        # mask folded into exp: exp(scale*s + ln(policy))
                        nc.scalar.activation(p_t, ps_s, AF.Exp,
                                             bias=lnp_sb[:, t:t + 1],
                                             scale=SCALE)
                        # software pipeline: O matmuls trail by one k-tile
                        if prev is not None:
                            o_mms(*prev)
                        prev = (ti, t, p_t)
                    o_mms(*prev)
                    for hi, ps_o in ((h0, ps_o0), (h1, ps_o1)):
                        # copy PSUM out immediately so the accumulator bank is
                        # released for the next pair
                        nc.vector.tensor_copy(o_keep[hi], ps_o)
                        # collect the denominator row into dall[hi] (tiny
                        # SBUF->SBUF DMA crosses partitions; off-engine)
                        dti, dri = _dall_slot(hi)
                        nc.sync.dma_start(
                            out=dall[dti][dri:dri + 1, :],
                            in_=o_keep[hi][HD:HD + 1, :])
                    # normalize ready heads in batches (one Ln+Exp each:
                    # 1/d = exp(-ln(d))) so most OTp tiles are finished while
                    # later pairs still run, and proj is barely gated at the
                    # end. partition_broadcast only reads partition 0, so
                    # each batch is flattened onto one partition first.
                    for hlo, hn in norm_sched.get(jp, ()):
                        rall = pB.tile([6, QB], bf16, name="rall",
                                       tag="rall", bufs=2)
                        nc.scalar.activation(
                            rall[0:hn, :], dall[_dall_slot(hlo)[0]][0:hn, :],
                            AF.Ln)
                        nc.scalar.activation(rall[0:hn, :], rall[0:hn, :],
                                             AF.Exp, scale=-1.0)
                        rflat = pB.tile([1, 6 * QB], bf16, name="rflat",
                                        tag="rflat", bufs=1)
                        nc.sync.dma_start(
                            out=rflat.rearrange(
                                "p (h q) -> p h q", h=6)[:, 0:hn, :],
                            in_=rall[0:hn, :])
                        for h in range(hlo, hlo + hn):
                            jph, hh = h // 2, (h % 2) * 64
                            bcd = pB.tile([64, QB], bf16, name="bcd",
                                          tag="bcd", bufs=2)
                            nc.gpsimd.partition_broadcast(
                                bcd, rflat[:, (h - hlo) * QB:
                                           (h - hlo + 1) * QB])
                            nc.vector.tensor_tensor(
                                out=OTp[jph][hh:hh + 64, :],
                                in0=o_keep[h][0:HD, :], in1=bcd, op=OP.mult)

            # proj in its own PSUM scope (psB's S/O banks are freed), LN2
            # stats share the scope so there is no pool barrier before them
            pC = pAB    # phase-C SBUF shares the pAB pool (fits comfortably)
            if not ln2_triv:
                ln2gb_sb = pC.tile([128, 2 * CT], f32, name="ln2gb_sb")
                nc.sync.dma_start(out=ln2gb_sb, in_=ln2gb_d.ap())
            h2T = [pC.tile([128, QB], bf16, name=f"h2t{cj}")
                   for cj in range(CT)]
            with tc.tile_pool(name="psP", bufs=1, space="PSUM") as psP:
                # proj, transposed: x_resT[cj] = sum_fj projwT[fj, cj].T @
                # OTp[fj] + xownT[cj]. The fj<5 partial sums are issued
                # first so the PE chews through them while the last head
                # pair (OTp[5]) is still being normalized.
                ps_pr = [psP.tile([128, QB], f32, name=f"ps_pr{cj}",
                                  tag=f"pspr{cj}", bufs=1)
                         for cj in range(CT)]
                for cj in range(CT):
                    for fj in range(FT - 1):
                        nc.tensor.matmul(
                            ps_pr[cj], projw[fj][:, cj * 128:(cj + 1) * 128],
                            OTp[fj],
                            start=(fj == 0), stop=False,
                            skip_group_check=True)
                # normalize the last two heads now (their denominators became
                # available at the end of pair 5); [1, QB] tiles sit at
                # partition 0 so no flatten hop is needed
                for h in (10, 11):
                    ralln = pC.tile([1, QB], bf16, name=f"ralln{h}",
                                    tag="ralln", bufs=2)
                    nc.scalar.activation(ralln, dall[2 + h - 10], AF.Ln)
                    nc.scalar.activation(ralln, ralln, AF.Exp, scale=-1.0)
                    bcd2 = pC.tile([64, QB], bf16, name="bcd2",
                                   tag="bcd2", bufs=2)
                    nc.gpsimd.partition_broadcast(bcd2, ralln)
                    jph, hh = h // 2, (h % 2) * 64
                    nc.vector.tensor_tensor(
                        out=OTp[jph][hh:hh + 64, :],
                        in0=o_keep[h][0:HD, :], in1=bcd2, op=OP.mult)
                for cj in range(CT):
                    nc.tensor.matmul(
                        ps_pr[cj], projw[FT - 1][:, cj * 128:(cj + 1) * 128],
                        OTp[FT - 1],
                        start=False, stop=True, skip_group_check=True)
                    if projb_triv:
                        nc.vector.tensor_add(x_resT[cj], ps_pr[cj],
                                             xownT[cj])
                    else:
                        nc.vector.tensor_add(x_resT[cj], ps_pr[cj],
                                             xownT[cj])
                        nc.vector.tensor_scalar(
                            x_resT[cj], x_resT[cj].bitcast(f32),
                            projb_sb[:, cj:cj + 1], 0.0,
                            op0=OP.add, op1=OP.add)

                # LN2 (transposed: stats over c via ones-matmuls), still in
                # the proj PSUM scope so proj(cj+1) overlaps stats(cj)
                ps_m2 = psP.tile([1, QB], f32, name="ps_m2")
                ps_sq2 = psP.tile([1, QB], f32, name="ps_sq2")
                for cj in range(CT):
                    nc.tensor.matmul(ps_m2, ones_fr, x_resT[cj],
                                     start=(cj == 0), stop=(cj == CT - 1))
                for cj in range(CT):
                    xsq2 = pC.tile([128, QB], f32r, name="xsq2", tag="xsq2",
                                   bufs=2)
                    nc.vector.tensor_mul(xsq2, x_resT[cj].bitcast(f32),
                                         x_resT[cj].bitcast(f32))
                    nc.tensor.matmul(ps_sq2, ones_fr, xsq2,
                                     start=(cj == 0), stop=(cj == CT - 1))
                m2row = pC.tile([1, QB], f32, name="m2row")
                nc.vector.tensor_scalar_mul(m2row, ps_m2, 1.0 / C)
                ve2 = pC.tile([1, QB], f32, name="ve2")
                nc.vector.tensor_scalar(ve2, ps_sq2, 1.0 / C, LN_EPS,
                                        op0=OP.mult, op1=OP.add)
                m2sq = pC.tile([1, QB], f32, name="m2sq")
                nc.vector.tensor_mul(m2sq, m2row, m2row)
                nc.vector.tensor_sub(ve2, ve2, m2sq)
                nc.scalar.activation(ve2, ve2, AF.Ln)
                r2row = pC.tile([1, QB], f32, name="r2row")
                nc.scalar.activation(r2row, ve2, AF.Exp, scale=-0.5)
                m216 = pC.tile([1, QB], bf16, name="m216")
                nc.vector.tensor_copy(m216, m2row)
                r216 = pC.tile([1, QB], bf16, name="r216")
                nc.vector.tensor_copy(r216, r2row)
                # broadcast m/r on the PE, reusing two freed proj banks
                ps_bc2m = psP.tile([128, QB], f32, name="ps_bc2m",
                                   tag="pspr0", bufs=1)
                nc.tensor.matmul(ps_bc2m, ones_row, m216,
                                 start=True, stop=True)
                ps_bc2r = psP.tile([128, QB], f32, name="ps_bc2r",
                                   tag="pspr1", bufs=1)
                nc.tensor.matmul(ps_bc2r, ones_row, r216,
                                 start=True, stop=True)
                bc2_m = pC.tile([128, QB], f32, name="bc2_m")
                nc.vector.tensor_copy(bc2_m, ps_bc2m)
                bc2_r = pC.tile([128, QB], bf16, name="bc2_r")
                nc.vector.tensor_copy(bc2_r, ps_bc2r)
                for cj in range(CT):
                    d2 = pC.tile([128, QB], bf16, name="d2", tag="d2",
                                 bufs=2)
                    nc.vector.tensor_sub(d2, x_resT[cj].bitcast(f32), bc2_m)
                    if ln2_triv:
                        nc.vector.tensor_tensor(out=h2T[cj], in0=d2,
                                                in1=bc2_r, op=OP.mult)
                    else:
                        nc.vector.tensor_tensor(out=d2, in0=d2, in1=bc2_r,
                                                op=OP.mult)
                        nc.vector.tensor_scalar(
                            h2T[cj], d2, ln2gb_sb[:, cj:cj + 1],
                            ln2gb_sb[:, CT + cj:CT + cj + 1],
                            op0=OP.mult, op1=OP.add)

            # fc1 + gelu + fc2, interleaved per 128-hid block so the PE
            # streams fc1(hj) -> fc2(hj-1) with no drain between phases.
            with tc.tile_pool(name="psC2", bufs=1, space="PSUM") as psC2:
                ps_f2 = [psC2.tile([128, QB], f32, name=f"psf2_{cj}",
                                   tag=f"psf2_{cj}", bufs=1)
                         for cj in range(CT)]
                for hblk in range(6):
                    w1 = []
                    for cj in range(CT):
                        w1t = pC.tile([128, 512], bf16, name="w1",
                                      tag=f"w1_{cj}", bufs=2)
                        nc.sync.dma_start(out=w1t, in_=fc1w_d.ap()[cj, hblk])
                        w1.append(w1t)
                    for hl_ in range(4):
                        hj = hblk * 4 + hl_
                        ps_f1 = psC2.tile([128, QB], f32, name="ps_f1",
                                          tag="psf1", bufs=2)
                        for cj in range(CT):
                            nc.tensor.matmul(
                                ps_f1, w1[cj][:, hl_ * 128:(hl_ + 1) * 128],
                                h2T[cj], start=(cj == 0), stop=(cj == CT - 1))
                        gT = pC.tile([128, QB], bf16, name="gT", tag="gt",
                                     bufs=3)
                        nc.scalar.activation(gT, ps_f1, AF.Gelu,
                                             bias=fc1b_sb[:, hj:hj + 1])
                        w2 = pC.tile([128, C], bf16, name="w2", tag="w2",
                                     bufs=3)
                        nc.gpsimd.dma_start(
                            out=w2,
                            in_=fc2w_d.ap()[hj * 128:(hj + 1) * 128, :])
                        for cj in range(CT):
                            nc.tensor.matmul(
                                ps_f2[cj], w2[:, cj * 128:(cj + 1) * 128],
                                gT,
                                start=(hj == 0), stop=(hj == HB - 1),
                                skip_group_check=True)
                # final residual add on-device; fc2 bias is added on host
                for cj in range(CT):
                    out_t = pC.tile([128, QB], bf16, name="out_t", tag="outt",
                                    bufs=2)
                    nc.vector.tensor_add(out_t, ps_f2[cj],
                                         x_resT[cj].bitcast(f32))
                    nc.sync.dma_start(
                        out=yT_d.ap()[cj * 128:(cj + 1) * 128, :],
                        in_=out_t)

    # Prefer the combined natural_log_exp table set so the Ln/Exp mix in this
    # kernel resolves to ONE ACT table set (the default chooser picks
    # single-anchor sets and thrashes ~1.3us per switch).
    import concourse.bacc as _bacc_mod
    _orig_tables = _bacc_mod.get_activation_tables

    def _pref_tables(arch):
        t = _orig_tables(arch)
        out = {}
        for name, fns in t.items():
            if name != "natural_log_exp_and_others":
                fns = {f for f in fns if f not in (AF.Exp, AF.Ln)}
            out[name] = set(fns)
        return out

    _bacc_mod.get_activation_tables = _pref_tables
    try:
        nc.compile()
    finally:
        _bacc_mod.get_activation_tables = _orig_tables
    return nc


def _prep_shared(qkv_w, proj_w, fc1_w, fc2_w, fc1_b):
    """Host-side weight packing (shared across all cores), bf16."""
    bft = ml_dtypes.bfloat16
    qkvT = np.ascontiguousarray(qkv_w.T)          # [C, 3C]: q | k | v
    wq = qkvT[:, 0:C]
    wk = qkvT[:, C:2 * C]
    wv = qkvT[:, 2 * C:3 * C]

    def pack_T(w):
        # [C, F] -> [FT, 128, CT*128]; [fj, p, ci*128+f] = w[ci*128+p, fj*128+f]
        t = w.reshape(CT, 128, FT, 128)
        return np.ascontiguousarray(
            t.transpose(2, 1, 0, 3).reshape(FT, 128, CT * 128).astype(bft))

    wq_packT = pack_T(wq)
    wk_packT = pack_T(wk)
    wv_packT = np.ascontiguousarray(
        wv.reshape(CT, 128, C).transpose(1, 0, 2).reshape(128, CT * C)
        .astype(bft))
    projwT = np.ascontiguousarray(proj_w.T.astype(bft))      # [F, C]
    fc1T = np.ascontiguousarray(fc1_w.T)          # [C, HID]
    fc1_pack = np.empty((CT, 6, 128, 512), bft)
    for cj in range(CT):
        for hblk in range(6):
            fc1_pack[cj, hblk] = fc1T[cj * 128:(cj + 1) * 128,
                                      hblk * 512:(hblk + 1) * 512]
    fc2T = np.ascontiguousarray(fc2_w.T.astype(bft))          # [HID, C]
    fc1b_cols = np.ascontiguousarray(fc1_b.reshape(HB, 128).T)
    eye = np.eye(128, dtype=bft)
    return dict(wq_packT=wq_packT, wk_packT=wk_packT, wv_packT=wv_packT,
                projwT=projwT, fc1w_pack=fc1_pack, fc2wT=fc2T,
                fc1b=fc1b_cols, eye=eye)


def kernel(x, policy, ln1_g, ln1_b, qkv_w, proj_w, proj_b, ln2_g, ln2_b,
           fc1_w, fc1_b, fc2_w, fc2_b):
    global LAST_RESULTS
    bft = ml_dtypes.bfloat16
    x = np.asarray(x, np.float32)
    policy = np.asarray(policy, np.float32)

    ln1_triv = bool(np.all(ln1_g == 1.0) and np.all(ln1_b == 0.0))
    ln2_triv = bool(np.all(ln2_g == 1.0) and np.all(ln2_b == 0.0))
    projb_triv = bool(np.all(proj_b == 0.0))
    # key compaction: each core keeps its own 512 queries as keys 0:512 plus
    # all unmasked other keys; masked non-own keys never attend anywhere
    # (their post-mask P is ~e-50) so they are dropped from K/V entirely.
    pol2 = policy[:, :, 0] > 0.5
    cols_per_core = []
    for c in range(NCORES):
        b_, qoff = c // 4, (c % 4) * QB
        own = np.arange(qoff, qoff + QB)
        other = np.concatenate([np.arange(0, qoff), np.arange(qoff + QB, N)])
        other = other[pol2[b_, other]]
        cols_per_core.append(np.concatenate([own, other]))
    kmax = max(len(cl) for cl in cols_per_core)
    kpad = ((kmax + 511) // 512) * 512
    kt_b = (kmax + 127) // 128      # k-tiles with at least one real key

    key = (ln1_triv, ln2_triv, projb_triv, kpad, kt_b)
    if key not in _prog_cache:
        _prog_cache[key] = _build_program(*key)
    nc = _prog_cache[key]
    kt_n = kpad // 128

    shared = _prep_shared(np.asarray(qkv_w, np.float32),
                          np.asarray(proj_w, np.float32),
                          np.asarray(fc1_w, np.float32),
                          np.asarray(fc2_w, np.float32),
                          np.asarray(fc1_b, np.float32))
    if not ln1_triv:
        g = np.asarray(ln1_g, np.float32).reshape(CT, 128).T
        b = np.asarray(ln1_b, np.float32).reshape(CT, 128).T
        shared["ln1gb"] = np.ascontiguousarray(np.concatenate([g, b], axis=1))
    if not ln2_triv:
        g = np.asarray(ln2_g, np.float32).reshape(CT, 128).T
        b = np.asarray(ln2_b, np.float32).reshape(CT, 128).T
        shared["ln2gb"] = np.ascontiguousarray(np.concatenate([g, b], axis=1))
    if not projb_triv:
        shared["projb"] = np.ascontiguousarray(
            np.asarray(proj_b, np.float32).reshape(CT, 128).T)

    in_maps = []
    for c in range(NCORES):
        b_, qoff = c // 4, (c % 4) * QB
        cols = cols_per_core[c]
        xT_c = np.zeros((C, kpad), np.float32)
        xT_c[:, :len(cols)] = x[b_].T[:, cols]
        polp = np.zeros(kpad, np.float32)
        polp[:len(cols)] = policy[b_, cols, 0]
        lnp_cols = np.ascontiguousarray(
            np.where(polp > 0.5, 0.0, MASK_NEG).astype(np.float32)
            .reshape(kt_n, 128).T)
        m = dict(shared)
        m["xT"] = xT_c.astype(bft)
        m["xownT"] = np.ascontiguousarray(x[b_, qoff:qoff + QB].T)
        m["lnp"] = lnp_cols
        in_maps.append(m)

    res = run_bass_kernel_spmd(nc, in_maps, core_ids=list(range(NCORES)),
                               trace=TRACE, **TRACE_KWARGS)
    LAST_RESULTS = res
    out = np.empty((B, N, C), np.float32)
    fc2b_row = np.asarray(fc2_b, np.float32).reshape(1, C)
    for c in range(NCORES):
        b_, qoff = c // 4, (c % 4) * QB
        out[b_, qoff:qoff + QB] = (res.results[c]["yT"].T
                                   .astype(np.float32) + fc2b_row)
    return out



# revision 6
# speedup vs baseline: 1.2017x; 1.2017x over previous
"""Trainium2 Bass kernel for nn_Block_22497038696617 (dense transformer block).

Block: pre-LN attention with policy-masked softmax + pre-LN MLP (exact GELU).
  B=2, N=2048, C=768, H=12 heads x 64, HID=3072, fp32 in/out.

Sharding (8 cores, zero cross-core communication, single SPMD launch):
  core c -> batch b = c//4, query block qoff = (c%4)*512.
  Each core computes LN1 + K/V for the full (compacted) sequence of its
  batch, Q/attention/proj/MLP for its own 512 query rows, and writes its
  [C, 512] output slice (transposed). Host gathers + transposes.

Key compaction: attention is permutation-invariant over keys, and a key with
policy 0 contributes ~0 everywhere except its own query's diagonal. Each
core's key axis is [own 512 queries, unmasked other keys, pad], so the
diagonal exception lives in k-tiles 0..3 at column offset t*128 and the SPMD
program is identical on all cores.

v3 (this file) vs the bf16 v2 baseline (272.9us):
  * fp8(e4m3) DoubleRow matmuls for the K/V/Q projections, the attention
    O = V@P accumulation, and both MLP GEMMs: two 128-deep contraction
    subtiles per instruction at 0.5 cyc/row, i.e. 2x bf16 PE throughput.
    Weights are scaled x64 on the host so they sit in fp8's normal range
    (raw values ~1/sqrt(768) are subnormal in e4m3); the 1/64 unscale rides
    existing ACT copies (scale=) for free. End-to-end rel err ~1.2e-2
    (numpy-simulated 1.20e-2) vs the 2e-2 gate; S=K^T Q, the diag fix and
    proj stay bf16 since they cost little and keep S/x_res exact.
  * PSUM->SBUF copies for K/Q/V move from DVE to the ACT engine (idle in
    phase A), freeing the vector engine which otherwise gates phase A.
  * P (softmax numerators) is written by the exp directly as fp8 into
    k-tile-PAIR tiles [128, 2, 2*QB] so the O accumulation runs DoubleRow
    over key-tile pairs (5 pairs + 1 single for kt_b=11).
  * fc1/fc2 weights live in fp8 packs with c-tile / hid-tile pairs
    interleaved; gelu unscales fc1 via its activation scale and writes fp8
    gT pair tiles; the final residual add unscales fc2 via an ACT copy.
"""

from contextlib import ExitStack

import numpy as np
import ml_dtypes

import concourse.bacc as bacc
import concourse.mybir as mybir
import concourse.tile as tile
from concourse.bass_utils import run_bass_kernel_spmd

f32 = mybir.dt.float32
f32r = mybir.dt.float32r
bf16 = mybir.dt.bfloat16
f8 = mybir.dt.float8e4
AF = mybir.ActivationFunctionType
OP = mybir.AluOpType
DR = mybir.MatmulPerfMode.DoubleRow

B, N, C = 2, 2048, 768
H, HD = 12, 64
HID = 3072
NCORES = 8
QB = 512                 # own query rows per core
CT = C // 128            # 6 c-tiles
CP = CT // 2             # 3 c-tile pairs (DoubleRow)
FT = C // 128            # 6 f-tiles (H*HD == C)
HB = HID // 128          # 24 hid-tiles
HB2 = HB // 2            # 12 hid-tile pairs
SCALE = HD ** -0.5
LN_EPS = 1e-5
POL_EPS = 1e-6
MASK_NEG = -50.0
WS = 64.0                # fp8 weight scale (keeps weights out of subnormals)
IWS = 1.0 / WS

TRACE = False            # set True by the dev harness for profiling runs
TRACE_KWARGS = {}
LAST_RESULTS = None      # BassKernelResults of the last run (for timing)

_prog_cache = {}


def _build_program(ln1_triv, ln2_triv, projb_triv, kpad, kt_b):
    # kt_b = number of k-tiles containing any real key; K/V/S/O work beyond
    # kt_b*128 columns is skipped entirely
    kt_n = kpad // 128
    kq_n = (kt_b * 128 + 511) // 512     # 512-wide key chunks with real keys
    kb = kt_b * 128                      # real-key column count (128-aligned)
    # k-tile visit order for attention: diagonal tiles (0..3) last so the
    # pair's first exp isn't delayed during pipeline fill
    ORD = list(range(4, kt_b)) + [0, 1, 2, 3]
    NP = (kt_b + 1) // 2                 # key-tile pairs for DoubleRow O
    pair_of = {t: (pos // 2, pos % 2) for pos, t in enumerate(ORD)}
    nc = bacc.Bacc("TRN2", target_bir_lowering=False, debug=False,
                   num_devices=NCORES)

    # ---- DRAM I/O ----
    xT_d = nc.dram_tensor("xT", [C, kpad], bf16, kind="ExternalInput")
    xownT_d = nc.dram_tensor("xownT", [C, QB], f32, kind="ExternalInput")
    lnp_d = nc.dram_tensor("lnp", [128, kt_n], f32, kind="ExternalInput")
    # fp8 weight packs with contraction-tile PAIRS interleaved for DoubleRow:
    # wq/wk: [fj, p, j2*256 + i*128 + f] = w[(2*j2+i)*128+p, fj*128+f]*WS
    wq_d = nc.dram_tensor("wq_packT", [FT, 128, CP * 256], f8,
                          kind="ExternalInput")
    wk_d = nc.dram_tensor("wk_packT", [FT, 128, CP * 256], f8,
                          kind="ExternalInput")
    wv_d = nc.dram_tensor("wv_packT", [128, CT * C], f8,
                          kind="ExternalInput")
    projw_d = nc.dram_tensor("projwT", [C, C], bf16, kind="ExternalInput")
    fc1w_d = nc.dram_tensor("fc1w_pack", [CP, 6, 128, 1024], f8,
                            kind="ExternalInput")
    fc2w_d = nc.dram_tensor("fc2w_pack", [HB2, 128, 2 * C], f8,
                            kind="ExternalInput")
    fc1b_d = nc.dram_tensor("fc1b", [128, HB], f32, kind="ExternalInput")
    eye_d = nc.dram_tensor("eye", [128, 128], bf16, kind="ExternalInput")
    if not ln1_triv:
        ln1gb_d = nc.dram_tensor("ln1gb", [128, 2 * CT], f32,
                                 kind="ExternalInput")
    if not ln2_triv:
        ln2gb_d = nc.dram_tensor("ln2gb", [128, 2 * CT], f32,
                                 kind="ExternalInput")
    if not projb_triv:
        projb_d = nc.dram_tensor("projb", [128, CT], f32,
                                 kind="ExternalInput")
    yT_d = nc.dram_tensor("yT", [C, QB], bf16, kind="ExternalOutput")

    with tile.TileContext(nc) as tc, ExitStack() as ctx:
        # ---------------- constants + whole-kernel persistents --------------
        pG = ctx.enter_context(tc.tile_pool(name="pG", bufs=1))
        eye_sb = pG.tile([128, 128], bf16, name="eye_sb")
        nc.gpsimd.dma_start(out=eye_sb, in_=eye_d.ap())
        lnp_sb = pG.tile([128, kt_n], f32, name="lnp_sb")
        nc.gpsimd.dma_start(out=lnp_sb, in_=lnp_d.ap())
        fc1b_sb = pG.tile([128, HB], f32, name="fc1b_sb")
        nc.gpsimd.dma_start(out=fc1b_sb, in_=fc1b_d.ap())
        ones_bf = pG.tile([128, 1], bf16, name="ones_bf")
        nc.vector.memset(ones_bf, 1.0)
        ones_fr = pG.tile([128, 1], f32r, name="ones_fr")
        nc.vector.memset(ones_fr.bitcast(f32), 1.0)
        ones_row = pG.tile([1, 128], bf16, name="ones_row")
        nc.vector.memset(ones_row, 1.0)
        # diagonal-fix stationary tiles: diagd[t][k, j] = eye * (-lnp/SCALE)
        diagd = []
        for t in range(4):
            d_ = pG.tile([128, 128], bf16, name=f"diagd{t}")
            nc.vector.tensor_scalar(d_, eye_sb, lnp_sb[:, t:t + 1],
                                    -1.0 / SCALE, op0=OP.mult, op1=OP.mult)
            diagd.append(d_)
        # attention output, transposed, per head-pair: OTp[j] rows = features
        # of heads (2j, 2j+1), cols = own queries
        OTp = [pG.tile([128, QB], bf16, name=f"otp{j}") for j in range(FT)]
        # unnormalized per-head attention output + denominator row
        o_keep = [pG.tile([HD + 1, QB], bf16, name=f"okeep{h}")
                  for h in range(H)]
        # denominator rows grouped by normalization batch (engines can only
        # address APs based at partition 0, so one tile per batch; heads
        # 10/11 are normalized individually after the pair loop)
        dall = [pG.tile([6, QB], bf16, name="dall0"),
                pG.tile([6, QB], bf16, name="dall1"),
                pG.tile([1, QB], bf16, name="dall2"),
                pG.tile([1, QB], bf16, name="dall3")]
        _dall_slot = lambda h: ((0, h) if h < 6 else
                                (1, h - 6) if h < 10 else (2 + h - 10, 0))
        # attention-residual (transposed) lives here so proj can write it and
        # phase C can read it
        # f32r so the LN2 stats matmuls can consume it directly (the BIR
        # verifier requires f32r matmul inputs to be rounded at the producer)
        x_resT = [pG.tile([128, QB], f32r, name=f"xrest{cj}")
                  for cj in range(CT)]

        # ======================= phase A + B scope ==========================
        with tc.tile_pool(name="pAB", bufs=1) as pAB:
            KTp = [pAB.tile([128, kb], bf16, name=f"ktp{j}") for j in range(FT)]
            QTp = [pAB.tile([128, QB], bf16, name=f"qtp{j}") for j in range(FT)]
            # V in fp8, packed in key-tile PAIRS for DoubleRow O matmuls.
            # Per-subtile extent padded 780 -> 784: dual-fp8 LdWeights
            # requires the subtile stride to be a multiple of 16 bytes.
            VP = ((H * (HD + 1) + 15) // 16) * 16
            vpadp = [pAB.tile([128, 2, VP], f8, name=f"vpadp{j}")
                     for j in range(NP)]
            # resident K weights (single DMA, first on the gpsimd queue so
            # kvq(0) is never DMA-gated), then V; Q weights stream in pA
            wk_sb = [pAB.tile([128, CP, 2, 128], f8, name=f"wk{fj}")
                     for fj in range(FT)]
            for fj in range(FT):
                nc.gpsimd.dma_start(
                    out=wk_sb[fj].rearrange("p a b c -> p (a b c)"),
                    in_=wk_d.ap()[fj])
            wv_sb = pAB.tile([128, CT, C], f8, name="wv_sb")
            nc.gpsimd.dma_start(
                out=wv_sb.rearrange("p a b -> p (a b)"), in_=wv_d.ap())
            # pre-warm the partition_broadcast ucode library (~10us invisible
            # IRAM load that blocks the gpsimd queue): after the weight DMAs,
            # long before phase B needs the first real broadcast
            bwarm = pAB.tile([2, 1], f32, name="bwarm")
            nc.gpsimd.partition_broadcast(bwarm, ones_fr.bitcast(f32)[0:1, :])

            # --------------- phase A: LN1 + QKV projections -----------------
            # Software-pipelined: chunk q's LN stats/apply overlap chunk
            # q-1's K/V/Q matmuls.
            with tc.tile_pool(name="pA", bufs=1) as pA, \
                 tc.tile_pool(name="psA", bufs=1, space="PSUM") as psA:
                if not ln1_triv:
                    ln1gb_sb = pA.tile([128, 2 * CT], f32, name="ln1gb_sb")
                    nc.sync.dma_start(out=ln1gb_sb, in_=ln1gb_d.ap())

                def ln_loads_stats(qr):
                    """x.T load (one 3-D DMA) + stats matmuls for one chunk."""
                    s0 = qr * 512
                    kw = min(512, kb - s0)
                    xtt = pA.tile([128, CT, 512], bf16, name="xt", tag="xt",
                                  bufs=2)
                    nc.sync.dma_start(
                        out=xtt[:, :, 0:kw],
                        in_=xT_d.ap()[:, s0:s0 + kw].rearrange(
                            "(a p) k -> p a k", p=128))
                    xt = [xtt[:, ci, :] for ci in range(CT)]
                    # stats via ones-matmuls (sum over c = partition dim)
                    ps_mean = psA.tile([1, 512], f32, name="ps_mean",
                                       tag="psmean", bufs=1)
                    ps_sq = psA.tile([1, 512], f32, name="ps_sq",
                                     tag="pssq", bufs=1)
                    for ci in range(CT):
                        nc.tensor.matmul(ps_mean[:, 0:kw], ones_bf,
                                         xt[ci][:, 0:kw],
                                         start=(ci == 0), stop=(ci == CT - 1))
                    for ci in range(CT):
                        xsq = pA.tile([128, 512], bf16, name="xsq", tag="xsq",
                                      bufs=2)
                        nc.vector.tensor_mul(xsq[:, 0:kw], xt[ci][:, 0:kw],
                                             xt[ci][:, 0:kw])
                        nc.tensor.matmul(ps_sq[:, 0:kw], ones_bf,
                                         xsq[:, 0:kw],
                                         start=(ci == 0), stop=(ci == CT - 1))
                    return xt, ps_mean, ps_sq, kw

                def ln_rows_hl(qr, stage):
                    """LN1 row stats -> broadcast -> h_ln.T build (fp8)."""
                    xt, ps_mean, ps_sq, kw = stage
                    def row(nm, dt=f32):
                        return pA.tile([1, 512], dt, name=nm, tag="rows",
                                       bufs=4)
                    mrow = row("mrow")
                    nc.vector.tensor_scalar_mul(mrow[:, 0:kw],
                                                ps_mean[:, 0:kw], 1.0 / C)
                    ve = row("ve")
                    nc.vector.tensor_scalar(ve[:, 0:kw], ps_sq[:, 0:kw],
                                            1.0 / C, LN_EPS,
                                            op0=OP.mult, op1=OP.add)
                    m2 = row("m2")
                    nc.vector.tensor_mul(m2[:, 0:kw], mrow[:, 0:kw],
                                         mrow[:, 0:kw])
                    nc.vector.tensor_sub(ve[:, 0:kw], ve[:, 0:kw],
                                         m2[:, 0:kw])
                    nc.scalar.activation(ve[:, 0:kw], ve[:, 0:kw], AF.Ln)
                    r0 = row("r0")
                    nc.scalar.activation(r0[:, 0:kw], ve[:, 0:kw], AF.Exp,
                                         scale=-0.5)
                    m16 = pA.tile([1, 512], bf16, name="m16", tag="rows16",
                                  bufs=2)
                    nc.vector.tensor_copy(m16[:, 0:kw], mrow[:, 0:kw])
                    r16 = pA.tile([1, 512], bf16, name="r16", tag="rows16",
                                  bufs=2)
                    nc.vector.tensor_copy(r16[:, 0:kw], r0[:, 0:kw])
                    # broadcast m/r across partitions on the PE (ones-row
                    # outer product) -- the gpsimd broadcast ucode costs a
                    # ~10us queue-blocking library load on first use
                    ps_bc = psA.tile([128, 1024], f32, name="ps_bc",
                                     tag="psbc", bufs=1)
                    nc.tensor.matmul(ps_bc[:, 0:kw], ones_row, m16[:, 0:kw],
                                     start=True, stop=True)
                    nc.tensor.matmul(ps_bc[:, 512:512 + kw], ones_row,
                                     r16[:, 0:kw], start=True, stop=True)
                    bc_m = pA.tile([128, 512], bf16, name="bc_m", tag="bc_m",
                                   bufs=2)
                    nc.vector.tensor_copy(bc_m[:, 0:kw], ps_bc[:, 0:kw])
                    bc_r = pA.tile([128, 512], bf16, name="bc_r", tag="bc_r",
                                   bufs=2)
                    nc.vector.tensor_copy(bc_r[:, 0:kw],
                                          ps_bc[:, 512:512 + kw])
                    # h_ln.T in fp8, all c-tiles in ONE tile so the K/V/Q
                    # DoubleRow matmuls can address c-tile pairs
                    hlall = pA.tile([128, CT, 512], f8, name="hl", tag="hl",
                                    bufs=2)
                    for ci in range(CT):
                        htmp = pA.tile([128, 512], bf16, name="htmp",
                                       tag="htmp", bufs=2)
                        nc.vector.tensor_sub(htmp[:, 0:kw], xt[ci][:, 0:kw],
                                             bc_m[:, 0:kw])
                        if ln1_triv:
                            nc.vector.tensor_tensor(
                                out=hlall[:, ci, 0:kw], in0=htmp[:, 0:kw],
                                in1=bc_r[:, 0:kw], op=OP.mult)
                        else:
                            nc.vector.tensor_tensor(
                                out=htmp[:, 0:kw], in0=htmp[:, 0:kw],
                                in1=bc_r[:, 0:kw], op=OP.mult)
                            nc.vector.tensor_scalar(
                                hlall[:, ci, 0:kw], htmp[:, 0:kw],
                                ln1gb_sb[:, ci:ci + 1],
                                ln1gb_sb[:, CT + ci:CT + ci + 1],
                                op0=OP.mult, op1=OP.add)
                    return hlall, kw

                def kvq_stage(qr, hlkw):
                    """K/V (+Q for chunk 0) fp8 DoubleRow matmuls, one chunk.
                    PSUM->SBUF copies ride the ACT engine (idle in phase A)
                    and fold the 1/WS weight unscale into their scale."""
                    hlall, kw = hlkw
                    s0 = qr * 512
                    for fj in range(FT):
                        psk = psA.tile([128, 512], f32, name="psk",
                                       tag="pskv", bufs=3)
                        for j2 in range(CP):
                            nc.tensor.matmul(
                                psk[:, 0:kw],
                                wk_sb[fj][:, j2, :, :],
                                hlall[:, 2 * j2:2 * j2 + 2, 0:kw],
                                start=(j2 == 0), stop=(j2 == CP - 1),
                                perf_mode=DR)
                        nc.scalar.activation(KTp[fj][:, s0:s0 + kw],
                                             psk[:, 0:kw], AF.Copy, scale=IWS)
                    for si in range(kw // 128):
                        st = qr * 4 + si
                        pi, slot = pair_of[st]
                        for fc in range(2):
                            f0 = fc * 512
                            wsz = 512 if fc == 0 else 256
                            psv = psA.tile([128, 512], f32, name="psv",
                                           tag="pskv", bufs=3)
                            for j2 in range(CP):
                                nc.tensor.matmul(
                                    psv[:, 0:wsz],
                                    hlall[:, 2 * j2:2 * j2 + 2,
                                          si * 128:(si + 1) * 128],
                                    wv_sb[:, 2 * j2:2 * j2 + 2, f0:f0 + wsz],
                                    start=(j2 == 0), stop=(j2 == CP - 1),
                                    perf_mode=DR)
                            nh = wsz // HD
                            h0 = 0 if fc == 0 else 8
                            nc.scalar.activation(
                                vpadp[pi][:, slot,
                                          h0 * 65:(h0 + nh) * 65].rearrange(
                                    "p (h d) -> p h d", d=65)[:, :, 0:HD],
                                psv[:, 0:wsz].rearrange(
                                    "p (h d) -> p h d", h=nh),
                                AF.Copy, scale=IWS)
                        nc.vector.memset(
                            vpadp[pi][:, slot, 0:H * 65].rearrange(
                                "p (h d) -> p h d", d=65)[:, :, HD], 1.0)
                    if qr == 0:
                        # own queries are keys 0:512 => Q.T from chunk 0
                        for fj in range(FT):
                            wq_t = pA.tile([128, CP, 2, 128], f8, name="wq",
                                           tag="wq", bufs=2)
                            nc.sync.dma_start(
                                out=wq_t.rearrange("p a b c -> p (a b c)"),
                                in_=wq_d.ap()[fj])
                            psq = psA.tile([128, 512], f32, name="psq",
                                           tag="pskv", bufs=3)
                            for j2 in range(CP):
                                nc.tensor.matmul(
                                    psq,
                                    wq_t[:, j2, :, :],
                                    hlall[:, 2 * j2:2 * j2 + 2, :],
                                    start=(j2 == 0), stop=(j2 == CP - 1),
                                    perf_mode=DR)
                            nc.scalar.activation(QTp[fj], psq, AF.Copy,
                                                 scale=IWS)

                # 2-deep software pipeline (see baseline)
                stage = ln_loads_stats(0)
                hl_prev = ln_rows_hl(0, stage)
                for qr in range(1, kq_n):
                    stage = ln_loads_stats(qr)
                    kvq_stage(qr - 1, hl_prev)
                    hl_prev = ln_rows_hl(qr, stage)
                kvq_stage(kq_n - 1, hl_prev)

            # --------------- phase B: attention (head pairs) ----------------
            with tc.tile_pool(name="pB", bufs=1) as pB, \
                 tc.tile_pool(name="psB", bufs=1, space="PSUM") as psB:
                # proj inputs live in pAB (they outlive the pB scope: proj
                # runs in the psP scope after attention)
                projw = [pAB.tile([128, C], bf16, name=f"pjw{fj}")
                         for fj in range(FT)]
                for fj in range(FT):
                    nc.sync.dma_start(
                        out=projw[fj],
                        in_=projw_d.ap()[fj * 128:(fj + 1) * 128, :])
                xownT = [pAB.tile([128, QB], f32, name=f"xownt{cj}")
                         for cj in range(CT)]
                for cj in range(CT):
                    nc.sync.dma_start(
                        out=xownT[cj],
                        in_=xownT_d.ap()[cj * 128:(cj + 1) * 128, :])
                if not projb_triv:
                    projb_sb = pAB.tile([128, CT], f32, name="projb_sb")
                    nc.sync.dma_start(out=projb_sb, in_=projb_d.ap())
                # heads 0-5 after pair 3, 6-9 after pair 4 (emitted in pair
                # 5's epilogue slot would be too late; key them to jp=4), and
                # 10-11 right at the end. Batches never cross a dall tile.
                norm_sched = {3: ((0, 6),), 4: ((6, 4),)}
                for jp in range(H // 2):
                    h0, h1 = 2 * jp, 2 * jp + 1
                    ps_o0 = psB.tile([HD + 1, QB], f32, name="ps_o0",
                                     tag="pso", bufs=4)
                    ps_o1 = psB.tile([HD + 1, QB], f32, name="ps_o1",
                                     tag="pso", bufs=4)

                    def o_mms(pi, p2t, first, last):
                        # DoubleRow over the key-tile pair; the last (odd)
                        # pair of an odd kt_b is a single fp8 matmul on slot 0
                        if 2 * pi + 1 < kt_b:
                            nc.tensor.matmul(
                                ps_o0, vpadp[pi][:, :, h0 * 65:h0 * 65 + 65],
                                p2t[:, :, 0:QB],
                                start=first, stop=last,
                                perf_mode=DR, skip_group_check=True)
                            nc.tensor.matmul(
                                ps_o1, vpadp[pi][:, :, h1 * 65:h1 * 65 + 65],
                                p2t[:, :, QB:2 * QB],
                                start=first, stop=last,
                                perf_mode=DR, skip_group_check=True)
                        else:
                            nc.tensor.matmul(
                                ps_o0, vpadp[pi][:, 0, h0 * 65:h0 * 65 + 65],
                                p2t[:, 0, 0:QB],
                                start=first, stop=last,
                                skip_group_check=True)
                            nc.tensor.matmul(
                                ps_o1, vpadp[pi][:, 0, h1 * 65:h1 * 65 + 65],
                                p2t[:, 0, QB:2 * QB],
                                start=first, stop=last,
                                skip_group_check=True)

                    p2t_of = {}
                    pending = []     # completed pairs awaiting O matmuls
                    n_done = 0       # O-issued pair count
                    for pos, t in enumerate(ORD):
                        ps_s = psB.tile([128, 2 * QB], f32, name="ps_s",
                                        tag="pss", bufs=2)
                        nc.tensor.matmul(
                            ps_s[:, 0:QB],
                            KTp[jp][0:64, t * 128:(t + 1) * 128],
                            QTp[jp][0:64, :],
                            start=True, stop=(t >= 4), skip_group_check=True)
                        nc.tensor.matmul(
                            ps_s[:, QB:2 * QB],
                            KTp[jp][64:128, t * 128:(t + 1) * 128],
                            QTp[jp][64:128, :],
                            start=True, stop=(t >= 4), skip_group_check=True)
                        if t < 4:
                            # diagonal fix: +(-lnp/SCALE) on the self column
                            off = t * 128
                            nc.tensor.matmul(
                                ps_s[:, off:off + 128], diagd[t], eye_sb,
                                start=False, stop=True, skip_group_check=True)
                            nc.tensor.matmul(
                                ps_s[:, QB + off:QB + off + 128], diagd[t],
                                eye_sb,
                                start=False, stop=True, skip_group_check=True)
                        pi, slot = pos // 2, pos % 2
                        if slot == 0:
                            p2t_of[pi] = pB.tile([128, 2, 2 * QB], f8,
                                                 name="p2", tag="pt", bufs=3)
                        # mask folded into exp: exp(scale*s + ln(policy)),
                        # written as fp8 into the pair tile's slot
                        nc.scalar.activation(p2t_of[pi][:, slot, :], ps_s,
                                             AF.Exp,
                                             bias=lnp_sb[:, t:t + 1],
                                             scale=SCALE)
                        if slot == 1 or pos == kt_b - 1:
                            pending.append(pi)
                        # software pipeline: O matmuls trail by one k-tile
                        # pair so the PE never waits on the exp it just issued
                        if len(pending) > 1:
                            pi_r = pending.pop(0)
                            o_mms(pi_r, p2t_of.pop(pi_r), n_done == 0,
                                  pi_r == NP - 1)
                            n_done += 1
                    for pi_r in pending:
                        o_mms(pi_r, p2t_of.pop(pi_r), n_done == 0,
                              pi_r == NP - 1)
                        n_done += 1
                    for hi, ps_o in ((h0, ps_o0), (h1, ps_o1)):
                        # copy PSUM out immediately so the accumulator bank is
                        # released for the next pair
                        nc.vector.tensor_copy(o_keep[hi], ps_o)
                        # collect the denominator row into dall[hi] (tiny
                        # SBUF->SBUF DMA crosses partitions; off-engine)
                        dti, dri = _dall_slot(hi)
                        nc.sync.dma_start(
                            out=dall[dti][dri:dri + 1, :],
                            in_=o_keep[hi][HD:HD + 1, :])
                    # normalize ready heads in batches (one Ln+Exp each:
                    # 1/d = exp(-ln(d))) so most OTp tiles are finished while
                    # later pairs still run, and proj is barely gated at the
                    # end. partition_broadcast only reads partition 0, so
                    # each batch is flattened onto one partition first.
                    for hlo, hn in norm_sched.get(jp, ()):
                        rall = pB.tile([6, QB], bf16, name="rall",
                                       tag="rall", bufs=2)
                        nc.scalar.activation(
                            rall[0:hn, :], dall[_dall_slot(hlo)[0]][0:hn, :],
                            AF.Ln)
                        nc.scalar.activation(rall[0:hn, :], rall[0:hn, :],
                                             AF.Exp, scale=-1.0)
                        rflat = pB.tile([1, 6 * QB], bf16, name="rflat",
                                        tag="rflat", bufs=1)
                        nc.sync.dma_start(
                            out=rflat.rearrange(
                                "p (h q) -> p h q", h=6)[:, 0:hn, :],
                            in_=rall[0:hn, :])
                        for h in range(hlo, hlo + hn):
                            jph, hh = h // 2, (h % 2) * 64
                            bcd = pB.tile([64, QB], bf16, name="bcd",
                                          tag="bcd", bufs=2)
                            nc.gpsimd.partition_broadcast(
                                bcd, rflat[:, (h - hlo) * QB:
                                           (h - hlo + 1) * QB])
                            nc.vector.tensor_tensor(
                                out=OTp[jph][hh:hh + 64, :],
                                in0=o_keep[h][0:HD, :], in1=bcd, op=OP.mult)

            # proj in its own PSUM scope (psB's S/O banks are freed), LN2
            # stats share the scope so there is no pool barrier before them
            pC = pAB    # phase-C SBUF shares the pAB pool (fits comfortably)
            if not ln2_triv:
                ln2gb_sb = pC.tile([128, 2 * CT], f32, name="ln2gb_sb")
                nc.sync.dma_start(out=ln2gb_sb, in_=ln2gb_d.ap())
            # h2 (LN2 output) in fp8, all c-tiles in one tile for fc1
            # DoubleRow pair addressing
            h2all = pC.tile([128, CT, QB], f8, name="h2all")
            with tc.tile_pool(name="psP", bufs=1, space="PSUM") as psP:
                # proj, transposed: x_resT[cj] = sum_fj projwT[fj, cj].T @
                # OTp[fj] + xownT[cj]. The fj<5 partial sums are issued
                # first so the PE chews through them while the last head
                # pair (OTp[5]) is still being normalized.
                ps_pr = [psP.tile([128, QB], f32, name=f"ps_pr{cj}",
                                  tag=f"pspr{cj}", bufs=1)
                         for cj in range(CT)]
                for cj in range(CT):
                    for fj in range(FT - 1):
                        nc.tensor.matmul(
                            ps_pr[cj], projw[fj][:, cj * 128:(cj + 1) * 128],
                            OTp[fj],
                            start=(fj == 0), stop=False,
                            skip_group_check=True)
                # normalize the last two heads now (their denominators became
                # available at the end of pair 5); [1, QB] tiles sit at
                # partition 0 so no flatten hop is needed
                for h in (10, 11):
                    ralln = pC.tile([1, QB], bf16, name=f"ralln{h}",
                                    tag="ralln", bufs=2)
                    nc.scalar.activation(ralln, dall[2 + h - 10], AF.Ln)
                    nc.scalar.activation(ralln, ralln, AF.Exp, scale=-1.0)
                    bcd2 = pC.tile([64, QB], bf16, name="bcd2",
                                   tag="bcd2", bufs=2)
                    nc.gpsimd.partition_broadcast(bcd2, ralln)
                    jph, hh = h // 2, (h % 2) * 64
                    nc.vector.tensor_tensor(
                        out=OTp[jph][hh:hh + 64, :],
                        in0=o_keep[h][0:HD, :], in1=bcd2, op=OP.mult)
                for cj in range(CT):
                    nc.tensor.matmul(
                        ps_pr[cj], projw[FT - 1][:, cj * 128:(cj + 1) * 128],
                        OTp[FT - 1],
                        start=False, stop=True, skip_group_check=True)
                    if projb_triv:
                        nc.vector.tensor_add(x_resT[cj], ps_pr[cj],
                                             xownT[cj])
                    else:
                        nc.vector.tensor_add(x_resT[cj], ps_pr[cj],
                                             xownT[cj])
                        nc.vector.tensor_scalar(
                            x_resT[cj], x_resT[cj].bitcast(f32),
                            projb_sb[:, cj:cj + 1], 0.0,
                            op0=OP.add, op1=OP.add)

                # LN2 (transposed: stats over c via ones-matmuls), still in
                # the proj PSUM scope so proj(cj+1) overlaps stats(cj)
                ps_m2 = psP.tile([1, QB], f32, name="ps_m2")
                ps_sq2 = psP.tile([1, QB], f32, name="ps_sq2")
                for cj in range(CT):
                    nc.tensor.matmul(ps_m2, ones_fr, x_resT[cj],
                                     start=(cj == 0), stop=(cj == CT - 1))
                for cj in range(CT):
                    xsq2 = pC.tile([128, QB], f32r, name="xsq2", tag="xsq2",
                                   bufs=2)
                    nc.vector.tensor_mul(xsq2, x_resT[cj].bitcast(f32),
                                         x_resT[cj].bitcast(f32))
                    nc.tensor.matmul(ps_sq2, ones_fr, xsq2,
                                     start=(cj == 0), stop=(cj == CT - 1))
                m2row = pC.tile([1, QB], f32, name="m2row")
                nc.vector.tensor_scalar_mul(m2row, ps_m2, 1.0 / C)
                ve2 = pC.tile([1, QB], f32, name="ve2")
                nc.vector.tensor_scalar(ve2, ps_sq2, 1.0 / C, LN_EPS,
                                        op0=OP.mult, op1=OP.add)
                m2sq = pC.tile([1, QB], f32, name="m2sq")
                nc.vector.tensor_mul(m2sq, m2row, m2row)
                nc.vector.tensor_sub(ve2, ve2, m2sq)
                nc.scalar.activation(ve2, ve2, AF.Ln)
                r2row = pC.tile([1, QB], f32, name="r2row")
                nc.scalar.activation(r2row, ve2, AF.Exp, scale=-0.5)
                m216 = pC.tile([1, QB], bf16, name="m216")
                nc.vector.tensor_copy(m216, m2row)
                r216 = pC.tile([1, QB], bf16, name="r216")
                nc.vector.tensor_copy(r216, r2row)
                # broadcast m/r on the PE, reusing two freed proj banks
                ps_bc2m = psP.tile([128, QB], f32, name="ps_bc2m",
                                   tag="pspr0", bufs=1)
                nc.tensor.matmul(ps_bc2m, ones_row, m216,
                                 start=True, stop=True)
                ps_bc2r = psP.tile([128, QB], f32, name="ps_bc2r",
                                   tag="pspr1", bufs=1)
                nc.tensor.matmul(ps_bc2r, ones_row, r216,
                                 start=True, stop=True)
                bc2_m = pC.tile([128, QB], f32, name="bc2_m")
                nc.vector.tensor_copy(bc2_m, ps_bc2m)
                bc2_r = pC.tile([128, QB], bf16, name="bc2_r")
                nc.vector.tensor_copy(bc2_r, ps_bc2r)
                for cj in range(CT):
                    d2 = pC.tile([128, QB], bf16, name="d2", tag="d2",
                                 bufs=2)
                    nc.vector.tensor_sub(d2, x_resT[cj].bitcast(f32), bc2_m)
                    if ln2_triv:
                        nc.vector.tensor_tensor(out=h2all[:, cj, :], in0=d2,
                                                in1=bc2_r, op=OP.mult)
                    else:
                        nc.vector.tensor_tensor(out=d2, in0=d2, in1=bc2_r,
                                                op=OP.mult)
                        nc.vector.tensor_scalar(
                            h2all[:, cj, :], d2, ln2gb_sb[:, cj:cj + 1],
                            ln2gb_sb[:, CT + cj:CT + cj + 1],
                            op0=OP.mult, op1=OP.add)

            # fc1 + gelu + fc2, fp8 DoubleRow, interleaved per hid-tile PAIR
            # so the PE streams fc1(hp) -> fc2(hp) with no drain between
            # phases. gelu unscales fc1 (scale=IWS); gT pair tiles feed fc2.
            with tc.tile_pool(name="psC2", bufs=1, space="PSUM") as psC2:
                ps_f2 = [psC2.tile([128, QB], f32, name=f"psf2_{cj}",
                                   tag=f"psf2_{cj}", bufs=1)
                         for cj in range(CT)]
                w1 = None
                for hp in range(HB2):
                    hblk = hp // 2
                    if hp % 2 == 0:
                        w1 = []
                        for cj2 in range(CP):
                            w1t = pC.tile([128, 2, 512], f8, name="w1",
                                          tag=f"w1_{cj2}", bufs=2)
                            nc.sync.dma_start(
                                out=w1t.rearrange("p a b -> p (a b)"),
                                in_=fc1w_d.ap()[cj2, hblk])
                            w1.append(w1t)
                    w2t = pC.tile([128, 2, C], f8, name="w2", tag="w2",
                                  bufs=3)
                    nc.gpsimd.dma_start(
                        out=w2t.rearrange("p a b -> p (a b)"),
                        in_=fc2w_d.ap()[hp])
                    gT2 = pC.tile([128, 2, QB], f8, name="gT2", tag="gt",
                                  bufs=3)
                    for l in range(2):
                        hj = 2 * hp + l
                        hl_ = hj % 4
                        ps_f1 = psC2.tile([128, QB], f32, name="ps_f1",
                                          tag="psf1", bufs=2)
                        for cj2 in range(CP):
                            nc.tensor.matmul(
                                ps_f1,
                                w1[cj2][:, :, hl_ * 128:(hl_ + 1) * 128],
                                h2all[:, 2 * cj2:2 * cj2 + 2, :],
                                start=(cj2 == 0), stop=(cj2 == CP - 1),
                                perf_mode=DR)
                        nc.scalar.activation(gT2[:, l, :], ps_f1, AF.Gelu,
                                             bias=fc1b_sb[:, hj:hj + 1],
                                             scale=IWS)
                    for cj in range(CT):
                        nc.tensor.matmul(
                            ps_f2[cj], w2t[:, :, cj * 128:(cj + 1) * 128],
                            gT2,
                            start=(hp == 0), stop=(hp == HB2 - 1),
                            perf_mode=DR, skip_group_check=True)
                # final residual add on-device; fc2 bias is added on host.
                # One fused DVE op: (psum * 1/WS) + x_res
                for cj in range(CT):
                    out_t = pC.tile([128, QB], bf16, name="out_t", tag="outt",
                                    bufs=2)
                    nc.vector.scalar_tensor_tensor(
                        out=out_t, in0=ps_f2[cj], scalar=IWS,
                        in1=x_resT[cj].bitcast(f32),
                        op0=OP.mult, op1=OP.add)
                    nc.sync.dma_start(
                        out=yT_d.ap()[cj * 128:(cj + 1) * 128, :],
                        in_=out_t)

    # Prefer the combined natural_log_exp table set so the Ln/Exp mix in this
    # kernel resolves to ONE ACT table set (the default chooser picks
    # single-anchor sets and thrashes ~1.3us per switch).
    import concourse.bacc as _bacc_mod
    _orig_tables = _bacc_mod.get_activation_tables

    def _pref_tables(arch):
        t = _orig_tables(arch)
        out = {}
        for name, fns in t.items():
            if name != "natural_log_exp_and_others":
                fns = {f for f in fns if f not in (AF.Exp, AF.Ln)}
            out[name] = set(fns)
        return out

    _bacc_mod.get_activation_tables = _pref_tables
    try:
        nc.compile()
    finally:
        _bacc_mod.get_activation_tables = _orig_tables
    return nc


def _prep_shared(qkv_w, proj_w, fc1_w, fc2_w, fc1_b):
    """Host-side weight packing (shared across all cores).

    fp8 packs are scaled by WS so the uniform(-1/sqrt(fan_in), ..) weights
    land in e4m3's normal range; consumers unscale via ACT copy/gelu scale."""
    bft = ml_dtypes.bfloat16
    f8t = ml_dtypes.float8_e4m3
    qkvT = np.ascontiguousarray(qkv_w.T)          # [C, 3C]: q | k | v
    wq = qkvT[:, 0:C]
    wk = qkvT[:, C:2 * C]
    wv = qkvT[:, 2 * C:3 * C]

    def pack_pair(w):
        # [C, F] -> [FT, 128, CP*256]; [fj, p, j2*256+i*128+f] =
        #   w[(2*j2+i)*128+p, fj*128+f] * WS
        t = (w * WS).reshape(CP, 2, 128, FT, 128)
        return np.ascontiguousarray(
            t.transpose(3, 2, 0, 1, 4).reshape(FT, 128, CP * 256)
            .astype(f8t))

    wq_packT = pack_pair(wq)
    wk_packT = pack_pair(wk)
    wv_packT = np.ascontiguousarray(
        (wv * WS).reshape(CT, 128, C).transpose(1, 0, 2).reshape(128, CT * C)
        .astype(f8t))
    projwT = np.ascontiguousarray(proj_w.T.astype(bft))      # [F, C]
    fc1T = np.ascontiguousarray(fc1_w.T)          # [C, HID]
    # [cj2, hblk, p, i*512+hcol] = fc1T[(2*cj2+i)*128+p, hblk*512+hcol]*WS
    fc1_pack = np.ascontiguousarray(
        (fc1T * WS).reshape(CP, 2, 128, 6, 512)
        .transpose(0, 3, 2, 1, 4).reshape(CP, 6, 128, 1024).astype(f8t))
    fc2T = np.ascontiguousarray(fc2_w.T)          # [HID, C]
    # [hp, p, i*C+c] = fc2T[(2*hp+i)*128+p, c]*WS
    fc2_pack = np.ascontiguousarray(
        (fc2T * WS).reshape(HB2, 2, 128, C)
        .transpose(0, 2, 1, 3).reshape(HB2, 128, 2 * C).astype(f8t))
    fc1b_cols = np.ascontiguousarray(fc1_b.reshape(HB, 128).T)
    eye = np.eye(128, dtype=bft)
    return dict(wq_packT=wq_packT, wk_packT=wk_packT, wv_packT=wv_packT,
                projwT=projwT, fc1w_pack=fc1_pack, fc2w_pack=fc2_pack,
                fc1b=fc1b_cols, eye=eye)


def kernel(x, policy, ln1_g, ln1_b, qkv_w, proj_w, proj_b, ln2_g, ln2_b,
           fc1_w, fc1_b, fc2_w, fc2_b):
    global LAST_RESULTS
    bft = ml_dtypes.bfloat16
    x = np.asarray(x, np.float32)
    policy = np.asarray(policy, np.float32)

    ln1_triv = bool(np.all(ln1_g == 1.0) and np.all(ln1_b == 0.0))
    ln2_triv = bool(np.all(ln2_g == 1.0) and np.all(ln2_b == 0.0))
    projb_triv = bool(np.all(proj_b == 0.0))
    # key compaction: each core keeps its own 512 queries as keys 0:512 plus
    # all unmasked other keys; masked non-own keys never attend anywhere
    # (their post-mask P is ~e-50) so they are dropped from K/V entirely.
    pol2 = policy[:, :, 0] > 0.5
    cols_per_core = []
    for c in range(NCORES):
        b_, qoff = c // 4, (c % 4) * QB
        own = np.arange(qoff, qoff + QB)
        other = np.concatenate([np.arange(0, qoff), np.arange(qoff + QB, N)])
        other = other[pol2[b_, other]]
        cols_per_core.append(np.concatenate([own, other]))
    kmax = max(len(cl) for cl in cols_per_core)
    kpad = ((kmax + 511) // 512) * 512
    kt_b = (kmax + 127) // 128      # k-tiles with at least one real key

    key = (ln1_triv, ln2_triv, projb_triv, kpad, kt_b)
    if key not in _prog_cache:
        _prog_cache[key] = _build_program(*key)
    nc = _prog_cache[key]
    kt_n = kpad // 128

    shared = _prep_shared(np.asarray(qkv_w, np.float32),
                          np.asarray(proj_w, np.float32),
                          np.asarray(fc1_w, np.float32),
                          np.asarray(fc2_w, np.float32),
                          np.asarray(fc1_b, np.float32))
    if not ln1_triv:
        g = np.asarray(ln1_g, np.float32).reshape(CT, 128).T
        b = np.asarray(ln1_b, np.float32).reshape(CT, 128).T
        shared["ln1gb"] = np.ascontiguousarray(np.concatenate([g, b], axis=1))
    if not ln2_triv:
        g = np.asarray(ln2_g, np.float32).reshape(CT, 128).T
        b = np.asarray(ln2_b, np.float32).reshape(CT, 128).T
        shared["ln2gb"] = np.ascontiguousarray(np.concatenate([g, b], axis=1))
    if not projb_triv:
        shared["projb"] = np.ascontiguousarray(
            np.asarray(proj_b, np.float32).reshape(CT, 128).T)

    in_maps = []
    for c in range(NCORES):
        b_, qoff = c // 4, (c % 4) * QB
        cols = cols_per_core[c]
        xT_c = np.zeros((C, kpad), np.float32)
        xT_c[:, :len(cols)] = x[b_].T[:, cols]
        polp = np.zeros(kpad, np.float32)
        polp[:len(cols)] = policy[b_, cols, 0]
        lnp_cols = np.ascontiguousarray(
            np.where(polp > 0.5, 0.0, MASK_NEG).astype(np.float32)
            .reshape(kt_n, 128).T)
        m = dict(shared)
        m["xT"] = xT_c.astype(bft)
        m["xownT"] = np.ascontiguousarray(x[b_, qoff:qoff + QB].T)
        m["lnp"] = lnp_cols
        in_maps.append(m)

    res = run_bass_kernel_spmd(nc, in_maps, core_ids=list(range(NCORES)),
                               trace=TRACE, **TRACE_KWARGS)
    LAST_RESULTS = res
    out = np.empty((B, N, C), np.float32)
    fc2b_row = np.asarray(fc2_b, np.float32).reshape(1, C)
    for c in range(NCORES):
        b_, qoff = c // 4, (c % 4) * QB
        out[b_, qoff:qoff + QB] = (res.results[c]["yT"].T
                                   .astype(np.float32) + fc2b_row)
    return out


# revision 21
# speedup vs baseline: 1.2220x; 1.0169x over previous
"""Trainium2 Bass kernel for nn_Block_22497038696617 (dense transformer block).

Block: pre-LN attention with policy-masked softmax + pre-LN MLP (exact GELU).
  B=2, N=2048, C=768, H=12 heads x 64, HID=3072, fp32 in/out.

Sharding (8 cores, zero cross-core communication, single SPMD launch):
  core c -> batch b = c//4, query block qoff = (c%4)*512.
  Each core computes LN1 + K/V for the full (compacted) sequence of its
  batch, Q/attention/proj/MLP for its own 512 query rows, and writes its
  [C, 512] output slice (transposed). Host gathers + transposes.

Key compaction: attention is permutation-invariant over keys, and a key with
policy 0 contributes ~0 everywhere except its own query's diagonal. Each
core's key axis is [own 512 queries, unmasked other keys, pad], so the
diagonal exception lives in k-tiles 0..3 at column offset t*128 and the SPMD
program is identical on all cores.

v3 (this file) vs the bf16 v2 baseline (272.9us):
  * fp8(e4m3) DoubleRow matmuls for the K/V/Q projections, the attention
    O = V@P accumulation, and both MLP GEMMs: two 128-deep contraction
    subtiles per instruction at 0.5 cyc/row, i.e. 2x bf16 PE throughput.
    Weights are scaled x64 on the host so they sit in fp8's normal range
    (raw values ~1/sqrt(768) are subnormal in e4m3); the 1/64 unscale rides
    existing ACT copies (scale=) for free. End-to-end rel err ~1.2e-2
    (numpy-simulated 1.20e-2) vs the 2e-2 gate; S=K^T Q, the diag fix and
    proj stay bf16 since they cost little and keep S/x_res exact.
  * PSUM->SBUF copies for K/Q/V move from DVE to the ACT engine (idle in
    phase A), freeing the vector engine which otherwise gates phase A.
  * P (softmax numerators) is written by the exp directly as fp8 into
    k-tile-PAIR tiles [128, 2, 2*QB] so the O accumulation runs DoubleRow
    over key-tile pairs (5 pairs + 1 single for kt_b=11).
  * fc1/fc2 weights live in fp8 packs with c-tile / hid-tile pairs
    interleaved; gelu unscales fc1 via its activation scale and writes fp8
    gT pair tiles; the final residual add unscales fc2 via an ACT copy.
"""

from contextlib import ExitStack

import numpy as np
import ml_dtypes

import concourse.bacc as bacc
import concourse.mybir as mybir
import concourse.tile as tile
from concourse.bass_utils import run_bass_kernel_spmd

f32 = mybir.dt.float32
f32r = mybir.dt.float32r
bf16 = mybir.dt.bfloat16
f8 = mybir.dt.float8e4
AF = mybir.ActivationFunctionType
OP = mybir.AluOpType
DR = mybir.MatmulPerfMode.DoubleRow

B, N, C = 2, 2048, 768
H, HD = 12, 64
HID = 3072
NCORES = 8
QB = 512                 # own query rows per core
CT = C // 128            # 6 c-tiles
CP = CT // 2             # 3 c-tile pairs (DoubleRow)
FT = C // 128            # 6 f-tiles (H*HD == C)
HB = HID // 128          # 24 hid-tiles
HB2 = HB // 2            # 12 hid-tile pairs
SCALE = HD ** -0.5
LN_EPS = 1e-5
POL_EPS = 1e-6
MASK_NEG = -50.0
WS = 64.0                # fp8 weight scale (keeps weights out of subnormals)
IWS = 1.0 / WS

TRACE = False            # set True by the dev harness for profiling runs
TRACE_KWARGS = {}
LAST_RESULTS = None      # BassKernelResults of the last run (for timing)

_prog_cache = {}


def _build_program(ln1_triv, ln2_triv, projb_triv, kpad, kt_b):
    # kt_b = number of k-tiles containing any real key; K/V/S/O work beyond
    # kt_b*128 columns is skipped entirely
    kt_n = kpad // 128
    kq_n = (kt_b * 128 + 511) // 512     # 512-wide key chunks with real keys
    kb = kt_b * 128                      # real-key column count (128-aligned)
    # k-tile visit order for attention: diagonal tiles (0..3) last so the
    # pair's first exp isn't delayed during pipeline fill
    ORD = list(range(4, kt_b)) + [0, 1, 2, 3]
    NP = (kt_b + 1) // 2                 # key-tile pairs for DoubleRow O
    pair_of = {t: (pos // 2, pos % 2) for pos, t in enumerate(ORD)}
    nc = bacc.Bacc("TRN2", target_bir_lowering=False, debug=False,
                   num_devices=NCORES)

    # ---- DRAM I/O ----
    xT_d = nc.dram_tensor("xT", [C, kpad], f8, kind="ExternalInput")
    xownT_d = nc.dram_tensor("xownT", [C, QB], f32, kind="ExternalInput")
    lnp_d = nc.dram_tensor("lnp", [128, kt_n], f32, kind="ExternalInput")
    # fp8 weight packs with contraction-tile PAIRS interleaved for DoubleRow:
    # wq/wk: [fj, p, j2*256 + i*128 + f] = w[(2*j2+i)*128+p, fj*128+f]*WS
    wq_d = nc.dram_tensor("wq_packT", [FT, 128, CP * 256], f8,
                          kind="ExternalInput")
    wk_d = nc.dram_tensor("wk_packT", [FT, 128, CP * 256], f8,
                          kind="ExternalInput")
    wv_d = nc.dram_tensor("wv_packT", [128, CT * C], f8,
                          kind="ExternalInput")
    projw_d = nc.dram_tensor("projwT", [C, C], bf16, kind="ExternalInput")
    fc1w_d = nc.dram_tensor("fc1w_pack", [128, CP * 6 * 1024], f8,
                            kind="ExternalInput")
    fc2w_d = nc.dram_tensor("fc2w_pack", [128, HB2 * 2 * C], f8,
                            kind="ExternalInput")
    fc1b_d = nc.dram_tensor("fc1b", [128, HB], f32, kind="ExternalInput")
    eye_d = nc.dram_tensor("eye", [128, 128], bf16, kind="ExternalInput")
    if not ln1_triv:
        ln1gb_d = nc.dram_tensor("ln1gb", [128, 2 * CT], f32,
                                 kind="ExternalInput")
    if not ln2_triv:
        ln2gb_d = nc.dram_tensor("ln2gb", [128, 2 * CT], f32,
                                 kind="ExternalInput")
    if not projb_triv:
        projb_d = nc.dram_tensor("projb", [128, CT], f32,
                                 kind="ExternalInput")
    yT_d = nc.dram_tensor("yT", [C, QB], bf16, kind="ExternalOutput")

    with tile.TileContext(nc) as tc, ExitStack() as ctx:
        # ---------------- constants + whole-kernel persistents --------------
        pG = ctx.enter_context(tc.tile_pool(name="pG", bufs=1))
        eye_sb = pG.tile([128, 128], bf16, name="eye_sb")
        nc.gpsimd.dma_start(out=eye_sb, in_=eye_d.ap())
        lnp_sb = pG.tile([128, kt_n], f32, name="lnp_sb")
        nc.gpsimd.dma_start(out=lnp_sb, in_=lnp_d.ap())
        fc1b_sb = pG.tile([128, HB], f32, name="fc1b_sb")
        nc.gpsimd.dma_start(out=fc1b_sb, in_=fc1b_d.ap())
        # fp8 ones pair-column for the DoubleRow LN1 stats matmuls; padded to
        # 16 free bytes per subtile (dual-fp8 LdWeights stride restriction)
        ones8 = pG.tile([128, 2, 16], f8, name="ones8")
        nc.vector.memset(ones8, 1.0)
        ones_fr = pG.tile([128, 1], f32r, name="ones_fr")
        nc.vector.memset(ones_fr.bitcast(f32), 1.0)
        ones_row = pG.tile([1, 128], bf16, name="ones_row")
        nc.vector.memset(ones_row, 1.0)
        # diagonal-fix stationary tiles: diagd[t][k, j] = eye * (-lnp/SCALE)
        diagd = []
        for t in range(4):
            d_ = pG.tile([128, 128], bf16, name=f"diagd{t}")
            nc.vector.tensor_scalar(d_, eye_sb, lnp_sb[:, t:t + 1],
                                    -1.0 / SCALE, op0=OP.mult, op1=OP.mult)
            diagd.append(d_)
        # attention output, transposed, per head-pair: OTp[j] rows = features
        # of heads (2j, 2j+1), cols = own queries
        OTp = [pG.tile([128, QB], bf16, name=f"otp{j}") for j in range(FT)]
        # unnormalized per-head attention output + denominator row
        o_keep = [pG.tile([HD + 1, QB], bf16, name=f"okeep{h}")
                  for h in range(H)]
        # denominator rows grouped by normalization batch (engines can only
        # address APs based at partition 0, so one tile per batch; heads
        # 10/11 are normalized individually after the pair loop)
        dall = [pG.tile([6, QB], bf16, name="dall0"),
                pG.tile([6, QB], bf16, name="dall1"),
                pG.tile([1, QB], bf16, name="dall2"),
                pG.tile([1, QB], bf16, name="dall3")]
        _dall_slot = lambda h: ((0, h) if h < 6 else
                                (1, h - 6) if h < 10 else (2 + h - 10, 0))
        # attention-residual (transposed) lives here so proj can write it and
        # phase C can read it
        # f32r so the LN2 stats matmuls can consume it directly (the BIR
        # verifier requires f32r matmul inputs to be rounded at the producer)
        x_resT = [pG.tile([128, QB], f32r, name=f"xrest{cj}")
                  for cj in range(CT)]

        # ======================= phase A + B scope ==========================
        with tc.tile_pool(name="pAB", bufs=1) as pAB:
            KTp = [pAB.tile([128, kb], bf16, name=f"ktp{j}") for j in range(FT)]
            QTp = [pAB.tile([128, QB], bf16, name=f"qtp{j}") for j in range(FT)]
            # V in fp8, packed in key-tile PAIRS for DoubleRow O matmuls.
            # Per-subtile extent padded 780 -> 784: dual-fp8 LdWeights
            # requires the subtile stride to be a multiple of 16 bytes.
            VP = ((H * (HD + 1) + 15) // 16) * 16
            vpadp = [pAB.tile([128, 2, VP], f8, name=f"vpadp{j}")
                     for j in range(NP)]
            # resident K weights (single DMA, first on the gpsimd queue so
            # kvq(0) is never DMA-gated), then V; Q weights stream in pA
            wk_sb = [pAB.tile([128, CP, 2, 128], f8, name=f"wk{fj}")
                     for fj in range(FT)]
            for fj in range(FT):
                nc.gpsimd.dma_start(
                    out=wk_sb[fj].rearrange("p a b c -> p (a b c)"),
                    in_=wk_d.ap()[fj])
            wv_sb = pAB.tile([128, CT, C], f8, name="wv_sb")
            nc.gpsimd.dma_start(
                out=wv_sb.rearrange("p a b -> p (a b)"), in_=wv_d.ap())
            # pre-warm the partition_broadcast ucode library (~10us invisible
            # IRAM load that blocks the gpsimd queue): after the weight DMAs,
            # long before phase B needs the first real broadcast
            bwarm = pAB.tile([2, 1], f32, name="bwarm")
            nc.gpsimd.partition_broadcast(bwarm, ones_fr.bitcast(f32)[0:1, :])

            # --------------- phase A: LN1 + QKV projections -----------------
            # Software-pipelined: chunk q's LN stats/apply overlap chunk
            # q-1's K/V/Q matmuls.
            with tc.tile_pool(name="pA", bufs=1) as pA, \
                 tc.tile_pool(name="psA", bufs=1, space="PSUM") as psA:
                if not ln1_triv:
                    ln1gb_sb = pA.tile([128, 2 * CT], f32, name="ln1gb_sb")
                    nc.sync.dma_start(out=ln1gb_sb, in_=ln1gb_d.ap())

                def ln_loads_stats(qr):
                    """x.T load (one fp8 DMA per c-tile pair, fine-grained
                    deps) + fp8 DoubleRow stats matmuls for one chunk."""
                    s0 = qr * 512
                    kw = min(512, kb - s0)
                    xtp = []
                    for j2 in range(CP):
                        xt8 = pA.tile([128, 2, 512], f8, name="xt",
                                      tag=f"xt{j2}", bufs=2)
                        nc.sync.dma_start(
                            out=xt8[:, :, 0:kw],
                            in_=xT_d.ap()[2 * j2 * 128:(2 * j2 + 2) * 128,
                                          s0:s0 + kw].rearrange(
                                "(a p) k -> p a k", p=128))
                        xtp.append(xt8)
                    # stats via ones-matmuls (sum over c = partition dim)
                    ps_mean = psA.tile([1, 512], f32, name="ps_mean",
                                       tag="psmean", bufs=1)
                    ps_sq = psA.tile([1, 512], f32, name="ps_sq",
                                     tag="pssq", bufs=1)
                    for j2 in range(CP):
                        nc.tensor.matmul(ps_mean[:, 0:kw], ones8[:, :, 0:1],
                                         xtp[j2][:, :, 0:kw],
                                         start=(j2 == 0), stop=(j2 == CP - 1),
                                         perf_mode=DR)
                    for j2 in range(CP):
                        xsq = pA.tile([128, 2, 512], f8, name="xsq",
                                      tag="xsq", bufs=2)
                        for i in range(2):
                            nc.vector.tensor_mul(xsq[:, i, 0:kw],
                                                 xtp[j2][:, i, 0:kw],
                                                 xtp[j2][:, i, 0:kw])
                        nc.tensor.matmul(ps_sq[:, 0:kw], ones8[:, :, 0:1],
                                         xsq[:, :, 0:kw],
                                         start=(j2 == 0), stop=(j2 == CP - 1),
                                         perf_mode=DR)
                    return xtp, ps_mean, ps_sq, kw

                def ln_rows_hl(qr, stage):
                    """LN1 row stats -> broadcast -> h_ln.T build (fp8)."""
                    xtp, ps_mean, ps_sq, kw = stage
                    def row(nm, dt=f32):
                        return pA.tile([1, 512], dt, name=nm, tag="rows",
                                       bufs=4)
                    mrow = row("mrow")
                    nc.vector.tensor_scalar_mul(mrow[:, 0:kw],
                                                ps_mean[:, 0:kw], 1.0 / C)
                    ve = row("ve")
                    nc.vector.tensor_scalar(ve[:, 0:kw], ps_sq[:, 0:kw],
                                            1.0 / C, LN_EPS,
                                            op0=OP.mult, op1=OP.add)
                    m2 = row("m2")
                    nc.vector.tensor_mul(m2[:, 0:kw], mrow[:, 0:kw],
                                         mrow[:, 0:kw])
                    nc.vector.tensor_sub(ve[:, 0:kw], ve[:, 0:kw],
                                         m2[:, 0:kw])
                    nc.scalar.activation(ve[:, 0:kw], ve[:, 0:kw], AF.Ln)
                    r0 = row("r0")
                    nc.scalar.activation(r0[:, 0:kw], ve[:, 0:kw], AF.Exp,
                                         scale=-0.5)
                    m16 = pA.tile([1, 512], bf16, name="m16", tag="rows16",
                                  bufs=2)
                    nc.vector.tensor_copy(m16[:, 0:kw], mrow[:, 0:kw])
                    r16 = pA.tile([1, 512], bf16, name="r16", tag="rows16",
                                  bufs=2)
                    nc.vector.tensor_copy(r16[:, 0:kw], r0[:, 0:kw])
                    # broadcast m/r across partitions on the PE (ones-row
                    # outer product) -- the gpsimd broadcast ucode costs a
                    # ~10us queue-blocking library load on first use
                    ps_bc = psA.tile([128, 1024], f32, name="ps_bc",
                                     tag="psbc", bufs=1)
                    nc.tensor.matmul(ps_bc[:, 0:kw], ones_row, m16[:, 0:kw],
                                     start=True, stop=True)
                    nc.tensor.matmul(ps_bc[:, 512:512 + kw], ones_row,
                                     r16[:, 0:kw], start=True, stop=True)
                    bc_m = pA.tile([128, 512], bf16, name="bc_m", tag="bc_m",
                                   bufs=2)
                    nc.vector.tensor_copy(bc_m[:, 0:kw], ps_bc[:, 0:kw])
                    bc_r = pA.tile([128, 512], bf16, name="bc_r", tag="bc_r",
                                   bufs=2)
                    nc.vector.tensor_copy(bc_r[:, 0:kw],
                                          ps_bc[:, 512:512 + kw])
                    # h_ln.T in fp8, one tile per c-tile PAIR so the K/V/Q
                    # DoubleRow matmuls get fine-grained dependencies
                    hlp = []
                    for j2 in range(CP):
                        hl8 = pA.tile([128, 2, 512], f8, name="hl",
                                      tag=f"hl{j2}", bufs=2)
                        for i in range(2):
                            ci = 2 * j2 + i
                            htmp = pA.tile([128, 512], bf16, name="htmp",
                                           tag="htmp", bufs=2)
                            nc.vector.tensor_sub(htmp[:, 0:kw],
                                                 xtp[j2][:, i, 0:kw],
                                                 bc_m[:, 0:kw])
                            if ln1_triv:
                                nc.vector.tensor_tensor(
                                    out=hl8[:, i, 0:kw], in0=htmp[:, 0:kw],
                                    in1=bc_r[:, 0:kw], op=OP.mult)
                            else:
                                nc.vector.tensor_tensor(
                                    out=htmp[:, 0:kw], in0=htmp[:, 0:kw],
                                    in1=bc_r[:, 0:kw], op=OP.mult)
                                nc.vector.tensor_scalar(
                                    hl8[:, i, 0:kw], htmp[:, 0:kw],
                                    ln1gb_sb[:, ci:ci + 1],
                                    ln1gb_sb[:, CT + ci:CT + ci + 1],
                                    op0=OP.mult, op1=OP.add)
                        hlp.append(hl8)
                    return hlp, kw

                def kvq_stage(qr, hlkw):
                    """K/V (+Q for chunk 0) fp8 DoubleRow matmuls, one chunk.
                    PSUM->SBUF copies ride the ACT engine (idle in phase A)
                    and fold the 1/WS weight unscale into their scale."""
                    hlp, kw = hlkw
                    s0 = qr * 512
                    for fj in range(FT):
                        psk = psA.tile([128, 512], f32, name="psk",
                                       tag="pskv", bufs=3)
                        for j2 in range(CP):
                            nc.tensor.matmul(
                                psk[:, 0:kw],
                                wk_sb[fj][:, j2, :, :],
                                hlp[j2][:, :, 0:kw],
                                start=(j2 == 0), stop=(j2 == CP - 1),
                                perf_mode=DR)
                        nc.scalar.activation(KTp[fj][:, s0:s0 + kw],
                                             psk[:, 0:kw], AF.Copy, scale=IWS)
                    for si in range(kw // 128):
                        st = qr * 4 + si
                        pi, slot = pair_of[st]
                        for fc in range(2):
                            f0 = fc * 512
                            wsz = 512 if fc == 0 else 256
                            psv = psA.tile([128, 512], f32, name="psv",
                                           tag="pskv", bufs=3)
                            for j2 in range(CP):
                                nc.tensor.matmul(
                                    psv[:, 0:wsz],
                                    hlp[j2][:, :, si * 128:(si + 1) * 128],
                                    wv_sb[:, 2 * j2:2 * j2 + 2, f0:f0 + wsz],
                                    start=(j2 == 0), stop=(j2 == CP - 1),
                                    perf_mode=DR)
                            nh = wsz // HD
                            h0 = 0 if fc == 0 else 8
                            nc.scalar.activation(
                                vpadp[pi][:, slot,
                                          h0 * 65:(h0 + nh) * 65].rearrange(
                                    "p (h d) -> p h d", d=65)[:, :, 0:HD],
                                psv[:, 0:wsz].rearrange(
                                    "p (h d) -> p h d", h=nh),
                                AF.Copy, scale=IWS)
                        nc.vector.memset(
                            vpadp[pi][:, slot, 0:H * 65].rearrange(
                                "p (h d) -> p h d", d=65)[:, :, HD], 1.0)
                    if qr == 0:
                        # own queries are keys 0:512 => Q.T from chunk 0
                        for fj in range(FT):
                            wq_t = pA.tile([128, CP, 2, 128], f8, name="wq",
                                           tag="wq", bufs=2)
                            nc.sync.dma_start(
                                out=wq_t.rearrange("p a b c -> p (a b c)"),
                                in_=wq_d.ap()[fj])
                            psq = psA.tile([128, 512], f32, name="psq",
                                           tag="pskv", bufs=3)
                            for j2 in range(CP):
                                nc.tensor.matmul(
                                    psq,
                                    wq_t[:, j2, :, :],
                                    hlp[j2],
                                    start=(j2 == 0), stop=(j2 == CP - 1),
                                    perf_mode=DR)
                            nc.scalar.activation(QTp[fj], psq, AF.Copy,
                                                 scale=IWS)

                # 2-deep software pipeline (see baseline)
                stage = ln_loads_stats(0)
                hl_prev = ln_rows_hl(0, stage)
                for qr in range(1, kq_n):
                    stage = ln_loads_stats(qr)
                    kvq_stage(qr - 1, hl_prev)
                    hl_prev = ln_rows_hl(qr, stage)
                kvq_stage(kq_n - 1, hl_prev)

            # --------------- phase B: attention (head pairs) ----------------
            with tc.tile_pool(name="pB", bufs=1) as pB, \
                 tc.tile_pool(name="psB", bufs=1, space="PSUM") as psB:
                # proj inputs live in pAB (they outlive the pB scope: proj
                # runs in the psP scope after attention)
                projw = [pAB.tile([128, C], bf16, name=f"pjw{fj}")
                         for fj in range(FT)]
                for fj in range(FT):
                    nc.sync.dma_start(
                        out=projw[fj],
                        in_=projw_d.ap()[fj * 128:(fj + 1) * 128, :])
                xownT = [pAB.tile([128, QB], f32, name=f"xownt{cj}")
                         for cj in range(CT)]
                for cj in range(CT):
                    nc.sync.dma_start(
                        out=xownT[cj],
                        in_=xownT_d.ap()[cj * 128:(cj + 1) * 128, :])
                if not projb_triv:
                    projb_sb = pAB.tile([128, CT], f32, name="projb_sb")
                    nc.sync.dma_start(out=projb_sb, in_=projb_d.ap())
                # bulk MLP weight loads (one DMA each) issued early on the
                # gpsimd queue so phase D never waits on weight streaming
                w1all = pAB.tile([128, CP, 6, 2, 512], f8, name="w1all")
                nc.gpsimd.dma_start(
                    out=w1all.rearrange("p a b c d -> p (a b c d)"),
                    in_=fc1w_d.ap())
                w2all = pAB.tile([128, HB2, 2, C], f8, name="w2all")
                nc.gpsimd.dma_start(
                    out=w2all.rearrange("p a b c -> p (a b c)"),
                    in_=fc2w_d.ap())
                # heads 0-5 after pair 3, 6-9 after pair 4 (emitted in pair
                # 5's epilogue slot would be too late; key them to jp=4), and
                # 10-11 right at the end. Batches never cross a dall tile.
                norm_sched = {3: ((0, 6),), 4: ((6, 4),)}
                for jp in range(H // 2):
                    h0, h1 = 2 * jp, 2 * jp + 1
                    ps_o0 = psB.tile([HD + 1, QB], f32, name="ps_o0",
                                     tag="pso", bufs=4)
                    ps_o1 = psB.tile([HD + 1, QB], f32, name="ps_o1",
                                     tag="pso", bufs=4)

                    def o_mms(pi, p2t, first, last):
                        # DoubleRow over the key-tile pair; the last (odd)
                        # pair of an odd kt_b is a single fp8 matmul on slot 0
                        if 2 * pi + 1 < kt_b:
                            nc.tensor.matmul(
                                ps_o0, vpadp[pi][:, :, h0 * 65:h0 * 65 + 65],
                                p2t[:, :, 0:QB],
                                start=first, stop=last,
                                perf_mode=DR, skip_group_check=True)
                            nc.tensor.matmul(
                                ps_o1, vpadp[pi][:, :, h1 * 65:h1 * 65 + 65],
                                p2t[:, :, QB:2 * QB],
                                start=first, stop=last,
                                perf_mode=DR, skip_group_check=True)
                        else:
                            nc.tensor.matmul(
                                ps_o0, vpadp[pi][:, 0, h0 * 65:h0 * 65 + 65],
                                p2t[:, 0, 0:QB],
                                start=first, stop=last,
                                skip_group_check=True)
                            nc.tensor.matmul(
                                ps_o1, vpadp[pi][:, 0, h1 * 65:h1 * 65 + 65],
                                p2t[:, 0, QB:2 * QB],
                                start=first, stop=last,
                                skip_group_check=True)

                    p2t_of = {}
                    pending = []     # completed pairs awaiting O matmuls
                    n_done = 0       # O-issued pair count
                    for pos, t in enumerate(ORD):
                        ps_s = psB.tile([128, 2 * QB], f32, name="ps_s",
                                        tag="pss", bufs=2)
                        nc.tensor.matmul(
                            ps_s[:, 0:QB],
                            KTp[jp][0:64, t * 128:(t + 1) * 128],
                            QTp[jp][0:64, :],
                            start=True, stop=(t >= 4), skip_group_check=True)
                        nc.tensor.matmul(
                            ps_s[:, QB:2 * QB],
                            KTp[jp][64:128, t * 128:(t + 1) * 128],
                            QTp[jp][64:128, :],
                            start=True, stop=(t >= 4), skip_group_check=True)
                        if t < 4:
                            # diagonal fix: +(-lnp/SCALE) on the self column
                            off = t * 128
                            nc.tensor.matmul(
                                ps_s[:, off:off + 128], diagd[t], eye_sb,
                                start=False, stop=True, skip_group_check=True)
                            nc.tensor.matmul(
                                ps_s[:, QB + off:QB + off + 128], diagd[t],
                                eye_sb,
                                start=False, stop=True, skip_group_check=True)
                        pi, slot = pos // 2, pos % 2
                        if slot == 0:
                            p2t_of[pi] = pB.tile([128, 2, 2 * QB], f8,
                                                 name="p2", tag="pt", bufs=3)
                        # mask folded into exp: exp(scale*s + ln(policy)),
                        # written as fp8 into the pair tile's slot
                        nc.scalar.activation(p2t_of[pi][:, slot, :], ps_s,
                                             AF.Exp,
                                             bias=lnp_sb[:, t:t + 1],
                                             scale=SCALE)
                        if slot == 1 or pos == kt_b - 1:
                            pending.append(pi)
                        # software pipeline: O matmuls trail by one k-tile
                        # pair so the PE never waits on the exp it just issued
                        if len(pending) > 1:
                            pi_r = pending.pop(0)
                            o_mms(pi_r, p2t_of.pop(pi_r), n_done == 0,
                                  pi_r == NP - 1)
                            n_done += 1
                    for pi_r in pending:
                        o_mms(pi_r, p2t_of.pop(pi_r), n_done == 0,
                              pi_r == NP - 1)
                        n_done += 1
                    for hi, ps_o in ((h0, ps_o0), (h1, ps_o1)):
                        # copy PSUM out immediately so the accumulator bank is
                        # released for the next pair
                        nc.vector.tensor_copy(o_keep[hi], ps_o)
                        # collect the denominator row into dall[hi] (tiny
                        # SBUF->SBUF DMA crosses partitions; off-engine)
                        dti, dri = _dall_slot(hi)
                        nc.sync.dma_start(
                            out=dall[dti][dri:dri + 1, :],
                            in_=o_keep[hi][HD:HD + 1, :])
                    # normalize ready heads in batches (one Ln+Exp each:
                    # 1/d = exp(-ln(d))) so most OTp tiles are finished while
                    # later pairs still run, and proj is barely gated at the
                    # end. partition_broadcast only reads partition 0, so
                    # each batch is flattened onto one partition first.
                    for hlo, hn in norm_sched.get(jp, ()):
                        rall = pB.tile([6, QB], bf16, name="rall",
                                       tag="rall", bufs=2)
                        nc.scalar.activation(
                            rall[0:hn, :], dall[_dall_slot(hlo)[0]][0:hn, :],
                            AF.Ln)
                        nc.scalar.activation(rall[0:hn, :], rall[0:hn, :],
                                             AF.Exp, scale=-1.0)
                        rflat = pB.tile([1, 6 * QB], bf16, name="rflat",
                                        tag="rflat", bufs=1)
                        nc.sync.dma_start(
                            out=rflat.rearrange(
                                "p (h q) -> p h q", h=6)[:, 0:hn, :],
                            in_=rall[0:hn, :])
                        for h in range(hlo, hlo + hn):
                            jph, hh = h // 2, (h % 2) * 64
                            bcd = pB.tile([64, QB], bf16, name="bcd",
                                          tag="bcd", bufs=2)
                            nc.gpsimd.partition_broadcast(
                                bcd, rflat[:, (h - hlo) * QB:
                                           (h - hlo + 1) * QB])
                            nc.vector.tensor_tensor(
                                out=OTp[jph][hh:hh + 64, :],
                                in0=o_keep[h][0:HD, :], in1=bcd, op=OP.mult)

            # proj in its own PSUM scope (psB's S/O banks are freed), LN2
            # stats share the scope so there is no pool barrier before them
            pC = pAB    # phase-C SBUF shares the pAB pool (fits comfortably)
            if not ln2_triv:
                ln2gb_sb = pC.tile([128, 2 * CT], f32, name="ln2gb_sb")
                nc.sync.dma_start(out=ln2gb_sb, in_=ln2gb_d.ap())
            # h2 (LN2 output) in fp8, one tile per c-tile pair for fc1
            # DoubleRow addressing with fine-grained dependencies
            h2p = [pC.tile([128, 2, QB], f8, name=f"h2p{j2}")
                   for j2 in range(CP)]
            with tc.tile_pool(name="psP", bufs=1, space="PSUM") as psP:
                # proj, transposed: x_resT[cj] = sum_fj projwT[fj, cj].T @
                # OTp[fj] + xownT[cj]. The fj<5 partial sums are issued
                # first so the PE chews through them while the last head
                # pair (OTp[5]) is still being normalized.
                ps_pr = [psP.tile([128, QB], f32, name=f"ps_pr{cj}",
                                  tag=f"pspr{cj}", bufs=1)
                         for cj in range(CT)]
                for cj in range(CT):
                    for fj in range(FT - 1):
                        nc.tensor.matmul(
                            ps_pr[cj], projw[fj][:, cj * 128:(cj + 1) * 128],
                            OTp[fj],
                            start=(fj == 0), stop=False,
                            skip_group_check=True)
                # normalize the last two heads now (their denominators became
                # available at the end of pair 5); [1, QB] tiles sit at
                # partition 0 so no flatten hop is needed
                for h in (10, 11):
                    ralln = pC.tile([1, QB], bf16, name=f"ralln{h}",
                                    tag="ralln", bufs=2)
                    nc.scalar.activation(ralln, dall[2 + h - 10], AF.Ln)
                    nc.scalar.activation(ralln, ralln, AF.Exp, scale=-1.0)
                    bcd2 = pC.tile([64, QB], bf16, name="bcd2",
                                   tag="bcd2", bufs=2)
                    nc.gpsimd.partition_broadcast(bcd2, ralln)
                    jph, hh = h // 2, (h % 2) * 64
                    nc.vector.tensor_tensor(
                        out=OTp[jph][hh:hh + 64, :],
                        in0=o_keep[h][0:HD, :], in1=bcd2, op=OP.mult)
                for cj in range(CT):
                    nc.tensor.matmul(
                        ps_pr[cj], projw[FT - 1][:, cj * 128:(cj + 1) * 128],
                        OTp[FT - 1],
                        start=False, stop=True, skip_group_check=True)
                    if projb_triv:
                        nc.vector.tensor_add(x_resT[cj], ps_pr[cj],
                                             xownT[cj])
                    else:
                        nc.vector.tensor_add(x_resT[cj], ps_pr[cj],
                                             xownT[cj])
                        nc.vector.tensor_scalar(
                            x_resT[cj], x_resT[cj].bitcast(f32),
                            projb_sb[:, cj:cj + 1], 0.0,
                            op0=OP.add, op1=OP.add)

                # LN2 (transposed: stats over c via ones-matmuls), still in
                # the proj PSUM scope so proj(cj+1) overlaps stats(cj)
                ps_m2 = psP.tile([1, QB], f32, name="ps_m2")
                ps_sq2 = psP.tile([1, QB], f32, name="ps_sq2")
                for cj in range(CT):
                    nc.tensor.matmul(ps_m2, ones_fr, x_resT[cj],
                                     start=(cj == 0), stop=(cj == CT - 1))
                for cj in range(CT):
                    xsq2 = pC.tile([128, QB], f32r, name="xsq2", tag="xsq2",
                                   bufs=2)
                    nc.vector.tensor_mul(xsq2, x_resT[cj].bitcast(f32),
                                         x_resT[cj].bitcast(f32))
                    nc.tensor.matmul(ps_sq2, ones_fr, xsq2,
                                     start=(cj == 0), stop=(cj == CT - 1))
                m2row = pC.tile([1, QB], f32, name="m2row")
                nc.vector.tensor_scalar_mul(m2row, ps_m2, 1.0 / C)
                ve2 = pC.tile([1, QB], f32, name="ve2")
                nc.vector.tensor_scalar(ve2, ps_sq2, 1.0 / C, LN_EPS,
                                        op0=OP.mult, op1=OP.add)
                m2sq = pC.tile([1, QB], f32, name="m2sq")
                nc.vector.tensor_mul(m2sq, m2row, m2row)
                nc.vector.tensor_sub(ve2, ve2, m2sq)
                nc.scalar.activation(ve2, ve2, AF.Ln)
                r2row = pC.tile([1, QB], f32, name="r2row")
                nc.scalar.activation(r2row, ve2, AF.Exp, scale=-0.5)
                m216 = pC.tile([1, QB], bf16, name="m216")
                nc.vector.tensor_copy(m216, m2row)
                r216 = pC.tile([1, QB], bf16, name="r216")
                nc.vector.tensor_copy(r216, r2row)
                # broadcast m/r on the PE, reusing two freed proj banks
                ps_bc2m = psP.tile([128, QB], f32, name="ps_bc2m",
                                   tag="pspr0", bufs=1)
                nc.tensor.matmul(ps_bc2m, ones_row, m216,
                                 start=True, stop=True)
                ps_bc2r = psP.tile([128, QB], f32, name="ps_bc2r",
                                   tag="pspr1", bufs=1)
                nc.tensor.matmul(ps_bc2r, ones_row, r216,
                                 start=True, stop=True)
                bc2_m = pC.tile([128, QB], f32, name="bc2_m")
                nc.vector.tensor_copy(bc2_m, ps_bc2m)
                bc2_r = pC.tile([128, QB], bf16, name="bc2_r")
                nc.vector.tensor_copy(bc2_r, ps_bc2r)
                for cj in range(CT):
                    d2 = pC.tile([128, QB], bf16, name="d2", tag="d2",
                                 bufs=2)
                    h2dst = h2p[cj // 2][:, cj % 2, :]
                    nc.vector.tensor_sub(d2, x_resT[cj].bitcast(f32), bc2_m)
                    if ln2_triv:
                        nc.vector.tensor_tensor(out=h2dst, in0=d2,
                                                in1=bc2_r, op=OP.mult)
                    else:
                        nc.vector.tensor_tensor(out=d2, in0=d2, in1=bc2_r,
                                                op=OP.mult)
                        nc.vector.tensor_scalar(
                            h2dst, d2, ln2gb_sb[:, cj:cj + 1],
                            ln2gb_sb[:, CT + cj:CT + cj + 1],
                            op0=OP.mult, op1=OP.add)

            # fc1 + gelu + fc2, fp8 DoubleRow, interleaved per hid-tile PAIR
            # so the PE streams fc1(hp) -> fc2(hp) with no drain between
            # phases. gelu unscales fc1 (scale=IWS); gT pair tiles feed fc2.
            with tc.tile_pool(name="psC2", bufs=1, space="PSUM") as psC2:
                ps_f2 = [psC2.tile([128, QB], f32, name=f"psf2_{cj}",
                                   tag=f"psf2_{cj}", bufs=1)
                         for cj in range(CT)]
                for hp in range(HB2):
                    hblk = hp // 2
                    gT2 = pC.tile([128, 2, QB], f8, name="gT2", tag="gt",
                                  bufs=3)
                    for l in range(2):
                        hj = 2 * hp + l
                        hl_ = hj % 4
                        ps_f1 = psC2.tile([128, QB], f32, name="ps_f1",
                                          tag="psf1", bufs=2)
                        for cj2 in range(CP):
                            nc.tensor.matmul(
                                ps_f1,
                                w1all[:, cj2, hblk, :,
                                      hl_ * 128:(hl_ + 1) * 128],
                                h2p[cj2],
                                start=(cj2 == 0), stop=(cj2 == CP - 1),
                                perf_mode=DR)
                        nc.scalar.activation(gT2[:, l, :], ps_f1, AF.Gelu,
                                             bias=fc1b_sb[:, hj:hj + 1],
                                             scale=IWS)
                    for cj in range(CT):
                        nc.tensor.matmul(
                            ps_f2[cj],
                            w2all[:, hp, :, cj * 128:(cj + 1) * 128],
                            gT2,
                            start=(hp == 0), stop=(hp == HB2 - 1),
                            perf_mode=DR, skip_group_check=True)
                # final residual add on-device; fc2 bias is added on host.
                # One fused DVE op: (psum * 1/WS) + x_res
                for cj in range(CT):
                    out_t = pC.tile([128, QB], bf16, name="out_t", tag="outt",
                                    bufs=2)
                    nc.vector.scalar_tensor_tensor(
                        out=out_t, in0=ps_f2[cj], scalar=IWS,
                        in1=x_resT[cj].bitcast(f32),
                        op0=OP.mult, op1=OP.add)
                    nc.sync.dma_start(
                        out=yT_d.ap()[cj * 128:(cj + 1) * 128, :],
                        in_=out_t)

    # Prefer the combined natural_log_exp table set so the Ln/Exp mix in this
    # kernel resolves to ONE ACT table set (the default chooser picks
    # single-anchor sets and thrashes ~1.3us per switch).
    import concourse.bacc as _bacc_mod
    _orig_tables = _bacc_mod.get_activation_tables

    def _pref_tables(arch):
        t = _orig_tables(arch)
        out = {}
        for name, fns in t.items():
            if name != "natural_log_exp_and_others":
                fns = {f for f in fns if f not in (AF.Exp, AF.Ln)}
            out[name] = set(fns)
        return out

    _bacc_mod.get_activation_tables = _pref_tables
    try:
        nc.compile()
    finally:
        _bacc_mod.get_activation_tables = _orig_tables
    return nc


def _prep_shared(qkv_w, proj_w, fc1_w, fc2_w, fc1_b):
    """Host-side weight packing (shared across all cores).

    fp8 packs are scaled by WS so the uniform(-1/sqrt(fan_in), ..) weights
    land in e4m3's normal range; consumers unscale via ACT copy/gelu scale."""
    bft = ml_dtypes.bfloat16
    f8t = ml_dtypes.float8_e4m3
    qkvT = np.ascontiguousarray(qkv_w.T)          # [C, 3C]: q | k | v
    wq = qkvT[:, 0:C]
    wk = qkvT[:, C:2 * C]
    wv = qkvT[:, 2 * C:3 * C]

    def pack_pair(w):
        # [C, F] -> [FT, 128, CP*256]; [fj, p, j2*256+i*128+f] =
        #   w[(2*j2+i)*128+p, fj*128+f] * WS
        t = (w * WS).reshape(CP, 2, 128, FT, 128)
        return np.ascontiguousarray(
            t.transpose(3, 2, 0, 1, 4).reshape(FT, 128, CP * 256)
            .astype(f8t))

    wq_packT = pack_pair(wq)
    wk_packT = pack_pair(wk)
    wv_packT = np.ascontiguousarray(
        (wv * WS).reshape(CT, 128, C).transpose(1, 0, 2).reshape(128, CT * C)
        .astype(f8t))
    projwT = np.ascontiguousarray(proj_w.T.astype(bft))      # [F, C]
    fc1T = np.ascontiguousarray(fc1_w.T)          # [C, HID]
    # partition-major: [p, cj2, hblk, i, hcol] =
    #   fc1T[(2*cj2+i)*128+p, hblk*512+hcol]*WS
    fc1_pack = np.ascontiguousarray(
        (fc1T * WS).reshape(CP, 2, 128, 6, 512)
        .transpose(2, 0, 3, 1, 4).reshape(128, CP * 6 * 1024).astype(f8t))
    fc2T = np.ascontiguousarray(fc2_w.T)          # [HID, C]
    # partition-major: [p, hp, i, c] = fc2T[(2*hp+i)*128+p, c]*WS
    fc2_pack = np.ascontiguousarray(
        (fc2T * WS).reshape(HB2, 2, 128, C)
        .transpose(2, 0, 1, 3).reshape(128, HB2 * 2 * C).astype(f8t))
    fc1b_cols = np.ascontiguousarray(fc1_b.reshape(HB, 128).T)
    eye = np.eye(128, dtype=bft)
    return dict(wq_packT=wq_packT, wk_packT=wk_packT, wv_packT=wv_packT,
                projwT=projwT, fc1w_pack=fc1_pack, fc2w_pack=fc2_pack,
                fc1b=fc1b_cols, eye=eye)


def kernel(x, policy, ln1_g, ln1_b, qkv_w, proj_w, proj_b, ln2_g, ln2_b,
           fc1_w, fc1_b, fc2_w, fc2_b):
    global LAST_RESULTS
    f8t = ml_dtypes.float8_e4m3
    x = np.asarray(x, np.float32)
    policy = np.asarray(policy, np.float32)

    ln1_triv = bool(np.all(ln1_g == 1.0) and np.all(ln1_b == 0.0))
    ln2_triv = bool(np.all(ln2_g == 1.0) and np.all(ln2_b == 0.0))
    projb_triv = bool(np.all(proj_b == 0.0))
    # key compaction: each core keeps its own 512 queries as keys 0:512 plus
    # all unmasked other keys; masked non-own keys never attend anywhere
    # (their post-mask P is ~e-50) so they are dropped from K/V entirely.
    pol2 = policy[:, :, 0] > 0.5
    cols_per_core = []
    for c in range(NCORES):
        b_, qoff = c // 4, (c % 4) * QB
        own = np.arange(qoff, qoff + QB)
        other = np.concatenate([np.arange(0, qoff), np.arange(qoff + QB, N)])
        other = other[pol2[b_, other]]
        cols_per_core.append(np.concatenate([own, other]))
    kmax = max(len(cl) for cl in cols_per_core)
    kpad = ((kmax + 511) // 512) * 512
    kt_b = (kmax + 127) // 128      # k-tiles with at least one real key

    key = (ln1_triv, ln2_triv, projb_triv, kpad, kt_b)
    if key not in _prog_cache:
        _prog_cache[key] = _build_program(*key)
    nc = _prog_cache[key]
    kt_n = kpad // 128

    shared = _prep_shared(np.asarray(qkv_w, np.float32),
                          np.asarray(proj_w, np.float32),
                          np.asarray(fc1_w, np.float32),
                          np.asarray(fc2_w, np.float32),
                          np.asarray(fc1_b, np.float32))
    if not ln1_triv:
        g = np.asarray(ln1_g, np.float32).reshape(CT, 128).T
        b = np.asarray(ln1_b, np.float32).reshape(CT, 128).T
        shared["ln1gb"] = np.ascontiguousarray(np.concatenate([g, b], axis=1))
    if not ln2_triv:
        g = np.asarray(ln2_g, np.float32).reshape(CT, 128).T
        b = np.asarray(ln2_b, np.float32).reshape(CT, 128).T
        shared["ln2gb"] = np.ascontiguousarray(np.concatenate([g, b], axis=1))
    if not projb_triv:
        shared["projb"] = np.ascontiguousarray(
            np.asarray(proj_b, np.float32).reshape(CT, 128).T)

    in_maps = []
    for c in range(NCORES):
        b_, qoff = c // 4, (c % 4) * QB
        cols = cols_per_core[c]
        xT_c = np.zeros((C, kpad), np.float32)
        xT_c[:, :len(cols)] = x[b_].T[:, cols]
        polp = np.zeros(kpad, np.float32)
        polp[:len(cols)] = policy[b_, cols, 0]
        lnp_cols = np.ascontiguousarray(
            np.where(polp > 0.5, 0.0, MASK_NEG).astype(np.float32)
            .reshape(kt_n, 128).T)
        m = dict(shared)
        m["xT"] = xT_c.astype(f8t)
        m["xownT"] = np.ascontiguousarray(x[b_, qoff:qoff + QB].T)
        m["lnp"] = lnp_cols
        in_maps.append(m)

    res = run_bass_kernel_spmd(nc, in_maps, core_ids=list(range(NCORES)),
                               trace=TRACE, **TRACE_KWARGS)
    LAST_RESULTS = res
    out = np.empty((B, N, C), np.float32)
    fc2b_row = np.asarray(fc2_b, np.float32).reshape(1, C)
    for c in range(NCORES):
        b_, qoff = c // 4, (c % 4) * QB
        out[b_, qoff:qoff + QB] = (res.results[c]["yT"].T
                                   .astype(np.float32) + fc2b_row)
    return out
